# revision 7
# baseline (speedup 1.0000x reference)
"""8-core Trainium2 Bass kernel for nn_MixModel (GCN mix model) — v3.

Sharding: nodes dealt round-robin by in-degree rank to 8 cores; each core owns
NLOC = ceil((ceil(N/8)+1)/128)*128 local rows (>=1 zero pad row reused as the
ELL gather-pad target).

Algebra used:
 - GCN messages factorize: msg = (h*dis)[src], output scaled by dis[dst]; the
   self-loop term is a local-tile add (pi-order stages) or an extra ELL slot
   (hop stage). Aggregation = unweighted padded-ELL gather+sum of pre-scaled
   table rows.
 - segsum and the layer matmul commute: sum((z@W*dis)[src]) =
   sum((z*dis)[src]) @ W — so cores AllGather the *scaled activations* and the
   per-layer matmul runs on the 98 aggregated dst tiles.
 - good/bad paths share edge sets -> gather concatenated 256-wide tables.
 - the permuted-input path's first-layer table is a cheap local permutation
   gather of the xW1' table (12.5k rows), not a per-edge pass.

Gather engine: gpsimd.dma_gather (InstDMAGatherAnt, mlp ucode library) with
int16 indices. The 100352-row shared tables exceed int16 range, so each ELL
tile is split into 4 chunk rectangles (chunk = 25088 consecutive table rows =
one core pair); chunk-local indices fit int16. Rectangles of consecutive tiles
are batched into one dma_gather call per chunk (<=32 j-columns per call,
<=96 per batch) amortizing the ~1us SWDGE fixed overhead over thousands of
row descriptors. Pad slots point at the chunk's zero pad row (local nloc-1).

Stages (per core):
  S0   xW1' shard = (x_sh @ W1) * dis_sh
  AG0  AllGather -> XW [NG,128]
  S2   T1 shard = [xW1'_loc | gather(XW, gperm)*ratio] ; AG1 -> T1 [NG,256]
  G1   ELL gather T1 -> zd = relu(dis^2 * sum)  (= z1*dis)      -> AG2 ZD
  G2   ELL gather ZD -> S ; e1{,b} = relu(dis * (S_h @ W2)) ;
       ship [e1*dish|e1b*dish] -> AG3a E1H ; [e1*dis] -> AG3b E1D ; e1 local
  G3   ELL gather E1H (hop order) -> embed2{,b} = dish * (S_h @ W3) -> E2h
  S12  MLP: embed3 = relu(e1@M1)@M2 ; tvec = embed3@Wd0
  S11  realign E2h to pi order ; scores = sigmoid(rowsum(tvec * e2{,b}))
  G4   ELL gather E1D -> cls = (dis*sum)@Wc -> OUT[:, :10]
"""

import numpy as np

import concourse.bacc as bacc
import concourse.bass as bass
import concourse.mybir as mybir
import concourse.tile as tile
from concourse import bass_utils
from concourse.masks import make_identity

P = 128
F32 = mybir.dt.float32
I32 = mybir.dt.int32
I16 = mybir.dt.int16
AF = mybir.ActivationFunctionType
ALU = mybir.AluOpType
TDT = mybir.dt.bfloat16  # transport/table dtype

JCALL = 8  # max j-columns per dma_gather call (num_idxs>1024 crashes the ucode)
JBATCH = 96  # max j-columns per SBUF gather tile


# ----------------------------------------------------------------- host prep


def _wrap16(flat):
    """flat [n*128] int array -> [128, 8*n] int16 16-wrap 8-replica layout."""
    n = len(flat)
    assert n % 128 == 0
    cols = n // 16
    arr = np.zeros((16, cols), np.int16)
    i = np.arange(n)
    arr[i % 16, i // 16] = flat
    return np.tile(arr, (8, 1))


def _ell_build_dg(src_g, dst_core, dst_loc, self_g, n_cores, nloc):
    """Chunked dma_gather ELL build.

    Returns (batches, idx16 per core [128, C]):
      batches: list of dict(
        col0: idx DRAM column offset of the batch,
        ncols: idx columns of the batch,
        jtot: j-columns in the batch's gather tile,
        calls: [(chunk, rel_col0, rel_ncols, j0, J)],
        tiles: [(t, [(j0, K), ...])]  # per-tile chunk rectangles (j0 rel)
      )
    """
    nt = nloc // P
    CH = 2 * nloc
    NCH = -(-(n_cores * nloc) // CH)
    padloc = nloc - 1

    if self_g is not None:
        # append self edges (dst (core, loc) <- self_g[core, loc])
        ac = np.repeat(np.arange(n_cores), nloc)
        al = np.tile(np.arange(nloc), n_cores)
        src_g = np.concatenate([src_g, self_g.ravel()])
        dst_core = np.concatenate([dst_core, ac])
        dst_loc = np.concatenate([dst_loc, al])

    ch = src_g // CH
    order = np.lexsort((ch, dst_loc, dst_core))
    sc, sl, sg, sch = dst_core[order], dst_loc[order], src_g[order], ch[order]
    slo = sg - sch * CH
    key = (sc * nloc + sl) * NCH + sch
    if len(key):
        is_start = np.r_[True, key[1:] != key[:-1]]
    else:
        is_start = np.array([], bool)
    run_starts = np.flatnonzero(is_start)
    run_len = np.diff(np.r_[run_starts, len(key)])
    pos_in_run = np.arange(len(key)) - np.repeat(run_starts, run_len)

    cnt = np.zeros((n_cores, nloc, NCH), np.int64)
    np.add.at(cnt, (sc, sl, sch), 1)
    # shared K per (tile, chunk): max over cores and lanes
    Ktc = cnt.reshape(n_cores, nt, P, NCH).max(axis=(0, 2))  # [nt, NCH]

    # batch assembly: greedy pack consecutive tiles
    batches = []
    t = 0
    while t < nt:
        tiles = []
        perch = np.zeros(NCH, np.int64)
        while t < nt:
            k = Ktc[t]
            if tiles and (perch + k).sum() > JBATCH:
                break
            perch += k
            tiles.append(t)
            t += 1
        # j layout: [c0 block | c1 block | ...]; within block, tiles in order
        jtot = int(perch.sum())
        calls = []
        tile_rects = {tt: [] for tt in tiles}
        j = 0
        for c in range(NCH):
            j0c = j
            for tt in tiles:
                K = int(Ktc[tt, c])
                tile_rects[tt].append((j, K))
                j += K
            # split the chunk block into <=JCALL-column calls (the ucode
            # can stage at most 128*JCALL indices per instruction)
            jj = j0c
            while jj < j:
                J = min(JCALL, j - jj)
                calls.append((c, 8 * jj, 8 * J, jj, J))
                jj += J
        batches.append(
            dict(
                jtot=jtot,
                calls=calls,
                tiles=[(tt, tile_rects[tt]) for tt in tiles],
            )
        )

    # per-core flat index arrays
    Jtotal = sum(b["jtot"] for b in batches)
    # global j offset per (tile, chunk) rectangle
    rect_j = np.zeros((nt, NCH), np.int64)
    B0 = 0
    for b in batches:
        b["col0"] = 8 * B0
        b["ncols"] = 8 * b["jtot"]
        for tt, rects in b["tiles"]:
            for c, (j0, K) in enumerate(rects):
                rect_j[tt, c] = B0 + j0
        B0 += b["jtot"]
    assert B0 == Jtotal

    idx_arrs = []
    for core in range(n_cores):
        flat = np.full(Jtotal * 128, padloc, np.int64)
        m = sc == core
        loc, pos, lo, cc = sl[m], pos_in_run[m], slo[m], sch[m]
        tt = loc // P
        p = loc % P
        j = rect_j[tt, cc] + pos
        flat[j * 128 + p] = lo
        assert flat.max() < 32768
        idx_arrs.append(_wrap16(flat))
    return batches, idx_arrs, int(Ktc.sum())


def _plane(vals_loc, nt):
    """[nloc] local-row vector -> [P, nt] plane (local row t*128+p -> [p, t])."""
    return np.ascontiguousarray(vals_loc.reshape(nt, P).T)


def prep(inputs, n_cores=8):
    x = np.asarray(inputs["x"], np.float32)
    ei = np.asarray(inputs["edge_index"], np.int64)
    eih = np.asarray(inputs["edge_index_hop"], np.int64)
    perm = np.asarray(inputs["perm"], np.int64)
    W1 = np.asarray(inputs["W1"], np.float32)
    W2 = np.asarray(inputs["W2"], np.float32)
    W3 = np.asarray(inputs["W3"], np.float32)
    M1 = np.asarray(inputs["M1"], np.float32)
    M2 = np.asarray(inputs["M2"], np.float32)
    Wc = np.asarray(inputs["Wc"], np.float32)
    Wd0 = np.asarray(inputs["Wd"], np.float32)[0]
    for bname in ("b1", "b2", "b3", "mb1", "mb2", "bc"):
        assert np.abs(np.asarray(inputs[bname])).max() == 0.0, (
            f"nonzero bias {bname} not supported by this kernel build"
        )

    N, n_feat = x.shape
    D = W1.shape[1]
    ncls = Wc.shape[1]
    max_real = -(-N // n_cores)
    nloc = -(-(max_real + 1) // P) * P
    nt = nloc // P
    ng = n_cores * nloc

    deg = np.bincount(ei[1], minlength=N).astype(np.float32) + 1.0
    degh = np.bincount(eih[1], minlength=N).astype(np.float32) + 1.0
    dis = 1.0 / np.sqrt(deg)
    dish = 1.0 / np.sqrt(degh)

    order = np.argsort(-deg, kind="stable")
    core_of = np.empty(N, np.int64)
    loc_of = np.empty(N, np.int64)
    core_of[order] = np.arange(N) % n_cores
    loc_of[order] = np.arange(N) // n_cores
    gl = core_of * nloc + loc_of
    padrow = [c * nloc + nloc - 1 for c in range(n_cores)]

    nat = np.full((n_cores, nloc), -1, np.int64)
    nat[core_of, loc_of] = np.arange(N)

    # hop order: per-core resort by hop degree desc (pads last)
    hkey = np.where(nat >= 0, -degh[np.maximum(nat, 0)], 1.0)
    hord = np.argsort(hkey, axis=1, kind="stable")
    hpos = np.argsort(hord, axis=1)

    selfg_pi = np.where(
        nat >= 0,
        np.arange(n_cores)[:, None] * nloc + np.arange(nloc)[None, :],
        np.array(padrow)[:, None],
    )
    batches1, idx1, nk1 = _ell_build_dg(
        gl[ei[0]], core_of[ei[1]], loc_of[ei[1]], None, n_cores, nloc
    )
    selfg_h = np.take_along_axis(selfg_pi, hord, axis=1)
    batches3, idx3, nk3 = _ell_build_dg(
        gl[eih[0]],
        core_of[eih[1]],
        hpos[core_of[eih[1]], loc_of[eih[1]]],
        selfg_h,
        n_cores,
        nloc,
    )

    in_maps = []
    for c in range(n_cores):
        natc = nat[c]
        real = natc >= 0
        xs = np.zeros((nloc, n_feat), np.float32)
        xs[real] = x[natc[real]]
        dis_c = np.ones(nloc, np.float32)
        dis_c[real] = dis[natc[real]]
        dish_pi = np.ones(nloc, np.float32)
        dish_pi[real] = dish[natc[real]]
        dishh = np.ones(nloc, np.float32)
        hnat = natc[hord[c]]
        hreal = hnat >= 0
        dishh[hreal] = dish[hnat[hreal]]
        gperm = np.full(nloc, padrow[c], np.int64)
        ratio = np.ones(nloc, np.float32)
        pv = perm[natc[real]]
        gperm[real] = gl[pv]
        ratio[real] = dis[natc[real]] / dis[pv]
        in_maps.append(
            {
                "xT": np.ascontiguousarray(xs.T),
                "dis_p": _plane(dis_c, nt),
                "dis2_p": _plane(dis_c * dis_c, nt),
                "dishp_p": _plane(dish_pi, nt),
                "dishh_p": _plane(dishh, nt),
                "ratio_p": _plane(ratio, nt),
                "gperm_p": _plane(gperm.astype(np.int32), nt),
                "idxR_p": _plane(hpos[c].astype(np.int32), nt),
                "idx1g": idx1[c],
                "idx3g": idx3[c],
                "W1": W1,
                "W2": W2,
                "W3": W3,
                "M1": M1,
                "M2": M2,
                "Wd0": Wd0,
                "Wc": np.ascontiguousarray(Wc),
            }
        )

    meta = dict(
        n_cores=n_cores,
        nloc=nloc,
        nt=nt,
        ng=ng,
        n_feat=n_feat,
        D=D,
        ncls=ncls,
        batches1=batches1,
        batches3=batches3,
        C1=idx1[0].shape[1],
        C3=idx3[0].shape[1],
        nk1=nk1,
        nk3=nk3,
        core_of=core_of,
        loc_of=loc_of,
    )
    return in_maps, meta


# ------------------------------------------------------------- device build


def build(meta):
    n_cores = meta["n_cores"]
    nloc, nt, ng = meta["nloc"], meta["nt"], meta["ng"]
    n_feat, D, ncls = meta["n_feat"], meta["D"], meta["ncls"]
    batches1, batches3 = meta["batches1"], meta["batches3"]
    C1, C3 = meta["C1"], meta["C3"]
    DD = 2 * D
    nfc = n_feat // P
    CH = 2 * nloc
    groups = [list(range(n_cores))]

    nc = bacc.Bacc("TRN2", debug=False, num_devices=n_cores)
    shared = "Shared" if n_cores > 4 else "Local"

    xT = nc.dram_tensor("xT", [n_feat, nloc], F32, kind="ExternalInput")
    dis_p = nc.dram_tensor("dis_p", [P, nt], F32, kind="ExternalInput")
    dis2_p = nc.dram_tensor("dis2_p", [P, nt], F32, kind="ExternalInput")
    dishp_p = nc.dram_tensor("dishp_p", [P, nt], F32, kind="ExternalInput")
    dishh_p = nc.dram_tensor("dishh_p", [P, nt], F32, kind="ExternalInput")
    ratio_p = nc.dram_tensor("ratio_p", [P, nt], F32, kind="ExternalInput")
    gperm_p = nc.dram_tensor("gperm_p", [P, nt], I32, kind="ExternalInput")
    idxR_p = nc.dram_tensor("idxR_p", [P, nt], I32, kind="ExternalInput")
    idx1g = nc.dram_tensor("idx1g", [P, C1], I16, kind="ExternalInput")
    idx3g = nc.dram_tensor("idx3g", [P, C3], I16, kind="ExternalInput")
    W1 = nc.dram_tensor("W1", [n_feat, D], F32, kind="ExternalInput")
    W2 = nc.dram_tensor("W2", [D, D], F32, kind="ExternalInput")
    W3 = nc.dram_tensor("W3", [D, D], F32, kind="ExternalInput")
    M1 = nc.dram_tensor("M1", [D, D], F32, kind="ExternalInput")
    M2 = nc.dram_tensor("M2", [D, D], F32, kind="ExternalInput")
    Wd0 = nc.dram_tensor("Wd0", [D, D], F32, kind="ExternalInput")
    Wc = nc.dram_tensor("Wc", [D, ncls], F32, kind="ExternalInput")
    out = nc.dram_tensor("out", [nloc, ncls + 2], F32, kind="ExternalOutput")

    xw_s = nc.dram_tensor("xw_s", [nloc, D], TDT, kind="Internal")
    XW = nc.dram_tensor("XW", [ng, D], TDT, kind="Internal", addr_space=shared)
    t1_s = nc.dram_tensor("t1_s", [nloc, DD], TDT, kind="Internal")
    T1 = nc.dram_tensor("T1", [ng, DD], TDT, kind="Internal", addr_space=shared)
    zd_s = nc.dram_tensor("zd_s", [nloc, DD], TDT, kind="Internal")
    ZD = nc.dram_tensor("ZD", [ng, DD], TDT, kind="Internal", addr_space=shared)
    e1_s = nc.dram_tensor("e1_s", [nloc, D], F32, kind="Internal")
    e1h_s = nc.dram_tensor("e1h_s", [nloc, DD], TDT, kind="Internal")
    e1d_s = nc.dram_tensor("e1d_s", [nloc, D], TDT, kind="Internal")
    E1H = nc.dram_tensor("E1H", [ng, DD], TDT, kind="Internal", addr_space=shared)
    E1D = nc.dram_tensor("E1D", [ng, D], TDT, kind="Internal", addr_space=shared)
    E2h = nc.dram_tensor("E2h", [nloc, DD], F32, kind="Internal")
    TV = nc.dram_tensor("TV", [nloc, D], F32, kind="Internal")

    with tile.TileContext(nc) as tc:
        with (
            tc.tile_pool(name="const", bufs=1) as constp,
            tc.tile_pool(name="gath", bufs=2) as gathp,
            tc.tile_pool(name="work", bufs=3) as workp,
            tc.tile_pool(name="outp", bufs=3) as outp,
            tc.tile_pool(name="psum", bufs=2, space="PSUM") as psp,
        ):
            ident = constp.tile([P, P], F32)
            make_identity(nc, ident[:])

            # resident planes + indices
            def res(t_dram, w, dt=F32, name=None):
                tl = constp.tile([P, w], dt, name=name)
                nc.sync.dma_start(tl[:], t_dram.ap())
                return tl

            disq = res(dis_p, nt, name="disq")
            dis2q = res(dis2_p, nt, name="dis2q")
            dishpq = res(dishp_p, nt, name="dishpq")
            dishhq = res(dishh_p, nt, name="dishhq")
            ratioq = res(ratio_p, nt, name="ratioq")
            gpermq = res(gperm_p, nt, I32, name="gpermq")
            idxRq = res(idxR_p, nt, I32, name="idxRq")

            w1t = [
                constp.tile([P, D], F32, name=f"w1t_{i}") for i in range(nfc)
            ]
            for i in range(nfc):
                nc.sync.dma_start(w1t[i][:], W1.ap()[i * P : (i + 1) * P])
            w2t = res(W2, D, name="w2t")
            w3t = res(W3, D, name="w3t")
            m1t = res(M1, D, name="m1t")
            m2t = res(M2, D, name="m2t")
            wdt = res(Wd0, D, name="wdt")
            wct = res(Wc, ncls, name="wct")

            def rows(t):
                return slice(t * P, (t + 1) * P)

            def col(plane, t):
                return plane[:, t : t + 1]

            # ---- S0: xW1' shard
            for t in range(nt):
                ps = psp.tile([P, D], F32, tag="mm")
                for i in range(nfc):
                    xt = workp.tile([P, P], F32, tag="xt")
                    nc.sync.dma_start(xt[:], xT.ap()[i * P : (i + 1) * P, rows(t)])
                    nc.tensor.matmul(
                        out=ps[:],
                        lhsT=xt[:],
                        rhs=w1t[i][:],
                        start=(i == 0),
                        stop=(i == nfc - 1),
                    )
                o = outp.tile([P, D], TDT, tag="s0")
                nc.vector.tensor_scalar_mul(o[:], ps[:], col(disq, t))
                nc.sync.dma_start(xw_s.ap()[rows(t)], o[:])

            nc.gpsimd.collective_compute(
                "AllGather",
                ALU.bypass,
                replica_groups=groups,
                ins=[xw_s[:].opt()],
                outs=[XW[:].opt()],
            )

            # ---- S2: T1 shard
            for t in range(nt):
                g = gathp.tile([P, D], TDT, tag="g2")
                nc.gpsimd.indirect_dma_start(
                    out=g[:],
                    out_offset=None,
                    in_=XW.ap(),
                    in_offset=bass.IndirectOffsetOnAxis(ap=col(gpermq, t), axis=0),
                )
                o = outp.tile([P, D], TDT, tag="s2")
                nc.vector.tensor_scalar_mul(o[:], g[:], col(ratioq, t))
                nc.sync.dma_start(t1_s.ap()[rows(t), D:DD], o[:])
                l = workp.tile([P, D], TDT, tag="s2l")
                nc.sync.dma_start(l[:], xw_s.ap()[rows(t)])
                nc.sync.dma_start(t1_s.ap()[rows(t), 0:D], l[:])

            nc.gpsimd.collective_compute(
                "AllGather",
                ALU.bypass,
                replica_groups=groups,
                ins=[t1_s[:].opt()],
                outs=[T1[:].opt()],
            )

            # ---- chunked-ELL gather driver -------------------------------
            # For each batch: load the batch's int16 index block, issue one
            # dma_gather per chunk, then per tile reduce its chunk rectangles
            # into an f32 [P, width] sum tile (callback receives (t, s)).
            def ell_run(table, width, batches, idxg, local_s, tail):
                for b in batches:
                    jtot = b["jtot"]
                    it = gathp.tile([P, b["ncols"]], I16, tag="ix")
                    nc.sync.dma_start(
                        it[:], idxg.ap()[:, b["col0"] : b["col0"] + b["ncols"]]
                    )
                    g = gathp.tile([P, jtot * width], TDT, tag="ge")
                    for c, rc0, rnc, j0, J in b["calls"]:
                        g3 = g[:, j0 * width : (j0 + J) * width].rearrange(
                            "p (j e) -> p j e", e=width
                        )
                        nc.gpsimd.dma_gather(
                            g3,
                            table.ap()[c * CH : (c + 1) * CH],
                            it[:, rc0 : rc0 + rnc],
                            J * P,
                            J * P,
                            width,
                        )
                    for t, rects in b["tiles"]:
                        s = workp.tile([P, width], F32, tag="se")
                        first = True
                        for j0, K in rects:
                            if K == 0:
                                continue
                            dst = s if first else workp.tile(
                                [P, width], F32, tag="sp"
                            )
                            sl_ = g[:, j0 * width : (j0 + K) * width]
                            if K == 1:
                                nc.vector.tensor_copy(dst[:], sl_)
                            else:
                                nc.vector.tensor_reduce(
                                    out=dst[:],
                                    in_=sl_.rearrange("p (k d) -> p d k", k=K),
                                    axis=mybir.AxisListType.X,
                                    op=ALU.add,
                                )
                            if not first:
                                nc.vector.tensor_tensor(
                                    out=s[:], in0=s[:], in1=dst[:], op=ALU.add
                                )
                            first = False
                        if local_s is not None:
                            li = workp.tile([P, width], TDT, tag="sl")
                            nc.sync.dma_start(li[:], local_s.ap()[rows(t)])
                            nc.vector.tensor_tensor(
                                out=s[:], in0=s[:], in1=li[:], op=ALU.add
                            )
                        tail(t, s)

            # ---- G1: zd = relu(dis2 * sum) -> zd_s
            def g1_tail(t, s):
                o = outp.tile([P, DD], TDT, tag="ze")
                nc.vector.tensor_scalar(
                    o[:], s[:], col(dis2q, t), 0.0, ALU.mult, ALU.max
                )
                nc.sync.dma_start(zd_s.ap()[rows(t)], o[:])

            ell_run(T1, DD, batches1, idx1g, t1_s, g1_tail)

            nc.gpsimd.collective_compute(
                "AllGather",
                ALU.bypass,
                replica_groups=groups,
                ins=[zd_s[:].opt()],
                outs=[ZD[:].opt()],
            )

            # ---- G2: S @ W2, three shipped variants
            def g2_tail(t, s):
                e1h = outp.tile([P, DD], TDT, tag="e1h")
                e1d = outp.tile([P, D], TDT, tag="e1d")
                e1p = outp.tile([P, D], F32, tag="e1p")
                for h in range(2):
                    tp = psp.tile([P, P], F32, tag="t", bufs=3)
                    nc.tensor.transpose(
                        out=tp[:], in_=s[:, h * D : (h + 1) * D], identity=ident[:]
                    )
                    tps = workp.tile([P, P], F32, tag="tps")
                    nc.vector.tensor_copy(tps[:], tp[:])
                    mm = psp.tile([P, D], F32, tag="m", bufs=3)
                    nc.tensor.matmul(
                        out=mm[:], lhsT=tps[:], rhs=w2t[:], start=True, stop=True
                    )
                    # e1 = relu(dis * mm)
                    eh = workp.tile([P, D], F32, tag="eh")
                    nc.vector.tensor_scalar(
                        eh[:], mm[:], col(disq, t), 0.0, ALU.mult, ALU.max
                    )
                    nc.vector.tensor_scalar_mul(
                        e1h[:, h * D : (h + 1) * D], eh[:], col(dishpq, t)
                    )
                    if h == 0:
                        nc.vector.tensor_copy(e1p[:], eh[:])
                        nc.vector.tensor_scalar_mul(e1d[:], eh[:], col(disq, t))
                nc.sync.dma_start(e1_s.ap()[rows(t)], e1p[:])
                nc.sync.dma_start(e1h_s.ap()[rows(t)], e1h[:])
                nc.sync.dma_start(e1d_s.ap()[rows(t)], e1d[:])

            ell_run(ZD, DD, batches1, idx1g, zd_s, g2_tail)

            nc.gpsimd.collective_compute(
                "AllGather",
                ALU.bypass,
                replica_groups=groups,
                ins=[e1h_s[:].opt()],
                outs=[E1H[:].opt()],
            )
            nc.gpsimd.collective_compute(
                "AllGather",
                ALU.bypass,
                replica_groups=groups,
                ins=[e1d_s[:].opt()],
                outs=[E1D[:].opt()],
            )

            # ---- S12: MLP + tvec (local, overlaps with AG3/G3)
            for t in range(nt):
                et = workp.tile([P, D], F32, tag="ml_in")
                nc.sync.dma_start(et[:], e1_s.ap()[rows(t)])
                tp = psp.tile([P, P], F32, tag="t", bufs=3)
                nc.tensor.transpose(out=tp[:], in_=et[:], identity=ident[:])
                tps = workp.tile([P, P], F32, tag="tps")
                nc.vector.tensor_copy(tps[:], tp[:])
                mm = psp.tile([P, D], F32, tag="m", bufs=3)
                nc.tensor.matmul(out=mm[:], lhsT=tps[:], rhs=m1t[:], start=True, stop=True)
                u = workp.tile([P, D], F32, tag="ml_u")
                nc.scalar.activation(u[:], mm[:], AF.Relu)
                tp2 = psp.tile([P, P], F32, tag="t", bufs=3)
                nc.tensor.transpose(out=tp2[:], in_=u[:], identity=ident[:])
                tps2 = workp.tile([P, P], F32, tag="tps")
                nc.vector.tensor_copy(tps2[:], tp2[:])
                mm2 = psp.tile([P, D], F32, tag="m", bufs=3)
                nc.tensor.matmul(
                    out=mm2[:], lhsT=tps2[:], rhs=m2t[:], start=True, stop=True
                )
                e3 = workp.tile([P, D], F32, tag="ml_e3")
                nc.vector.tensor_copy(e3[:], mm2[:])
                tp3 = psp.tile([P, P], F32, tag="t", bufs=3)
                nc.tensor.transpose(out=tp3[:], in_=e3[:], identity=ident[:])
                tps3 = workp.tile([P, P], F32, tag="tps")
                nc.vector.tensor_copy(tps3[:], tp3[:])
                mm3 = psp.tile([P, D], F32, tag="m", bufs=3)
                nc.tensor.matmul(
                    out=mm3[:], lhsT=tps3[:], rhs=wdt[:], start=True, stop=True
                )
                tv = outp.tile([P, D], F32, tag="ml_tv")
                nc.vector.tensor_copy(tv[:], mm3[:])
                nc.sync.dma_start(TV.ap()[rows(t)], tv[:])

            # ---- G3: embed2{,b} = dishh * (S_h @ W3) -> E2h (hop order)
            def g3_tail(t, s):
                e2 = outp.tile([P, DD], F32, tag="e2")
                for h in range(2):
                    tp = psp.tile([P, P], F32, tag="t", bufs=3)
                    nc.tensor.transpose(
                        out=tp[:], in_=s[:, h * D : (h + 1) * D], identity=ident[:]
                    )
                    tps = workp.tile([P, P], F32, tag="tps")
                    nc.vector.tensor_copy(tps[:], tp[:])
                    mm = psp.tile([P, D], F32, tag="m", bufs=3)
                    nc.tensor.matmul(
                        out=mm[:], lhsT=tps[:], rhs=w3t[:], start=True, stop=True
                    )
                    nc.vector.tensor_scalar_mul(
                        e2[:, h * D : (h + 1) * D], mm[:], col(dishhq, t)
                    )
                nc.sync.dma_start(E2h.ap()[rows(t)], e2[:])

            ell_run(E1H, DD, batches3, idx3g, None, g3_tail)

            # ---- S11 + S13: realign + scores
            for t in range(nt):
                e2 = gathp.tile([P, DD], F32, tag="gr")
                nc.gpsimd.indirect_dma_start(
                    out=e2[:],
                    out_offset=None,
                    in_=E2h.ap(),
                    in_offset=bass.IndirectOffsetOnAxis(ap=col(idxRq, t), axis=0),
                )
                tv = workp.tile([P, D], F32, tag="sc_tv")
                nc.sync.dma_start(tv[:], TV.ap()[rows(t)])
                pr = workp.tile([P, DD], F32, tag="sc_pr")
                nc.vector.tensor_mul(pr[:, 0:D], tv[:], e2[:, 0:D])
                nc.vector.tensor_mul(pr[:, D:DD], tv[:], e2[:, D:DD])
                rs = workp.tile([P, 2], F32, tag="sc_rs")
                nc.vector.tensor_reduce(
                    out=rs[:],
                    in_=pr[:].rearrange("p (h d) -> p h d", h=2),
                    axis=mybir.AxisListType.X,
                    op=ALU.add,
                )
                sg = outp.tile([P, 2], F32, tag="sc_sg")
                nc.scalar.activation(sg[:], rs[:], AF.Sigmoid)
                nc.sync.dma_start(out.ap()[rows(t), ncls : ncls + 2], sg[:])

            # ---- G4: cls = (dis * sum) @ Wc -> out[:, :ncls]
            def g4_tail(t, s):
                sc_ = workp.tile([P, D], F32, tag="c_s")
                nc.vector.tensor_scalar_mul(sc_[:], s[:], col(disq, t))
                tp = psp.tile([P, P], F32, tag="t", bufs=3)
                nc.tensor.transpose(out=tp[:], in_=sc_[:], identity=ident[:])
                tps = workp.tile([P, P], F32, tag="tps")
                nc.vector.tensor_copy(tps[:], tp[:])
                mm = psp.tile([P, ncls], F32, tag="m", bufs=3)
                nc.tensor.matmul(out=mm[:], lhsT=tps[:], rhs=wct[:], start=True, stop=True)
                o = outp.tile([P, ncls], F32, tag="c_o")
                nc.vector.tensor_copy(o[:], mm[:])
                nc.sync.dma_start(out.ap()[rows(t), 0:ncls], o[:])

            ell_run(E1D, D, batches1, idx1g, e1d_s, g4_tail)

    nc.compile()
    return nc


def assemble(results, meta):
    n_cores = meta["n_cores"]
    N = len(meta["core_of"])
    ncls = meta["ncls"]
    out = np.empty((N, ncls + 2), np.float32)
    for c in range(n_cores):
        oc = results[c]["out"]
        m = meta["core_of"] == c
        out[m] = oc[meta["loc_of"][m]]
    return out


# ------------------------------------------------------------------ entry


_CACHE = {}
TRACE = False
LAST_RES = None


def kernel(**inputs):
    """Full-input entry point: shards across 8 NeuronCores internally.

    Expects the nn_MixModel input dict (x, edge_index, edge_index_hop, y,
    perm, W1..Wd); returns the full [N, n_cls+2] float32 output.
    """
    n_cores = 8
    in_maps, meta = prep(inputs, n_cores)
    key = (meta["nloc"], meta["C1"], meta["C3"], meta["nk1"], meta["nk3"])
    nc = _CACHE.get(key)
    if nc is None:
        nc = build(meta)
        _CACHE[key] = nc
    res = bass_utils.run_bass_kernel_spmd(
        nc, in_maps, core_ids=list(range(n_cores)), trace=TRACE
    )
    global LAST_RES
    LAST_RES = res
    return assemble(res.results, meta)


# revision 9
# speedup vs baseline: 1.8052x; 1.8052x over previous
"""8-core Trainium2 Bass kernel for nn_MixModel (GCN mix model) — v3.

Sharding: nodes dealt round-robin by in-degree rank to 8 cores; each core owns
NLOC = ceil((ceil(N/8)+1)/128)*128 local rows (>=1 zero pad row reused as the
ELL gather-pad target).

Algebra used:
 - GCN messages factorize: msg = (h*dis)[src], output scaled by dis[dst]; the
   self-loop term is a local-tile add (pi-order stages) or an extra ELL slot
   (hop stage). Aggregation = unweighted padded-ELL gather+sum of pre-scaled
   table rows.
 - segsum and the layer matmul commute: sum((z@W*dis)[src]) =
   sum((z*dis)[src]) @ W — so cores AllGather the *scaled activations* and the
   per-layer matmul runs on the 98 aggregated dst tiles.
 - good/bad paths share edge sets -> gather concatenated 256-wide tables.
 - the permuted-input path's first-layer table is a cheap local permutation
   gather of the xW1' table (12.5k rows), not a per-edge pass.

Gather engine: gpsimd.dma_gather (InstDMAGatherAnt, mlp ucode library) with
int16 indices. The 100352-row shared tables exceed int16 range, so each ELL
tile is split into 4 chunk rectangles (chunk = 25088 consecutive table rows =
one core pair); chunk-local indices fit int16. Rectangles of consecutive tiles
are batched into one dma_gather call per chunk (<=32 j-columns per call,
<=96 per batch) amortizing the ~1us SWDGE fixed overhead over thousands of
row descriptors. Pad slots point at the chunk's zero pad row (local nloc-1).

Stages (per core):
  S0   xW1' shard = (x_sh @ W1) * dis_sh
  AG0  AllGather -> XW [NG,128]
  S2   T1 shard = [xW1'_loc | gather(XW, gperm)*ratio] ; AG1 -> T1 [NG,256]
  G1   ELL gather T1 -> zd = relu(dis^2 * sum)  (= z1*dis)      -> AG2 ZD
  G2   ELL gather ZD -> S ; e1{,b} = relu(dis * (S_h @ W2)) ;
       ship [e1*dish|e1b*dish] -> AG3a E1H ; [e1*dis] -> AG3b E1D ; e1 local
  G3   ELL gather E1H (hop order) -> embed2{,b} = dish * (S_h @ W3) -> E2h
  S12  MLP: embed3 = relu(e1@M1)@M2 ; tvec = embed3@Wd0
  S11  realign E2h to pi order ; scores = sigmoid(rowsum(tvec * e2{,b}))
  G4   ELL gather E1D -> cls = (dis*sum)@Wc -> OUT[:, :10]
"""

import numpy as np

import concourse.bacc as bacc
import concourse.bass as bass
import concourse.mybir as mybir
import concourse.tile as tile
from concourse import bass_utils
from concourse.masks import make_identity

P = 128
F32 = mybir.dt.float32
I32 = mybir.dt.int32
I16 = mybir.dt.int16
AF = mybir.ActivationFunctionType
ALU = mybir.AluOpType
TDT = mybir.dt.bfloat16  # transport/table dtype

# ----------------------------------------------------------------- host prep


def _ell_build(src_g, dst_core, dst_loc, self_g, n_cores, nloc, padrow):
    """Shared-K ELL: returns (K per tile, per-core int32 [P, sum(K)] arrays,
    p-major-global: element [p, koff[t]+k] = slot k of local row t*128+p)."""
    nt = nloc // P
    counts = np.zeros((n_cores, nloc), np.int64)
    np.add.at(counts, (dst_core, dst_loc), 1)
    n_self = 0 if self_g is None else 1
    cmax = counts.reshape(n_cores, nt, P).max(axis=(0, 2))
    K = (cmax + n_self).astype(np.int64)
    order = np.lexsort((dst_loc, dst_core))
    sc, sl, sg = dst_core[order], dst_loc[order], src_g[order]
    key = sc.astype(np.int64) * nloc + sl
    is_start = np.r_[True, key[1:] != key[:-1]] if len(key) else np.array([], bool)
    run_starts = np.flatnonzero(is_start)
    run_len = np.diff(np.r_[run_starts, len(key)])
    pos_in_run = np.arange(len(key)) - np.repeat(run_starts, run_len)
    koff = np.r_[0, np.cumsum(K)]
    sk = int(koff[-1])
    idx_arrs = []
    for c in range(n_cores):
        arr = np.full((P, sk), padrow[c], np.int64)
        m = sc == c
        loc, pos, gidx = sl[m], pos_in_run[m], sg[m]
        t = loc // P
        p = loc % P
        arr[p, koff[t] + pos + n_self] = gidx
        if n_self:
            allt = np.arange(nloc) // P
            allp = np.arange(nloc) % P
            arr[allp, koff[allt]] = self_g[c]
        idx_arrs.append(arr.astype(np.int32))
    return K.tolist(), idx_arrs


def _plane(vals_loc, nt):
    """[nloc] local-row vector -> [P, nt] plane (local row t*128+p -> [p, t])."""
    return np.ascontiguousarray(vals_loc.reshape(nt, P).T)


def prep(inputs, n_cores=8):
    x = np.asarray(inputs["x"], np.float32)
    ei = np.asarray(inputs["edge_index"], np.int64)
    eih = np.asarray(inputs["edge_index_hop"], np.int64)
    perm = np.asarray(inputs["perm"], np.int64)
    W1 = np.asarray(inputs["W1"], np.float32)
    W2 = np.asarray(inputs["W2"], np.float32)
    W3 = np.asarray(inputs["W3"], np.float32)
    M1 = np.asarray(inputs["M1"], np.float32)
    M2 = np.asarray(inputs["M2"], np.float32)
    Wc = np.asarray(inputs["Wc"], np.float32)
    Wd0 = np.asarray(inputs["Wd"], np.float32)[0]
    for bname in ("b1", "b2", "b3", "mb1", "mb2", "bc"):
        assert np.abs(np.asarray(inputs[bname])).max() == 0.0, (
            f"nonzero bias {bname} not supported by this kernel build"
        )

    N, n_feat = x.shape
    D = W1.shape[1]
    ncls = Wc.shape[1]
    max_real = -(-N // n_cores)
    nloc = -(-(max_real + 1) // P) * P
    nt = nloc // P
    ng = n_cores * nloc

    deg = np.bincount(ei[1], minlength=N).astype(np.float32) + 1.0
    degh = np.bincount(eih[1], minlength=N).astype(np.float32) + 1.0
    dis = 1.0 / np.sqrt(deg)
    dish = 1.0 / np.sqrt(degh)

    order = np.argsort(-deg, kind="stable")
    core_of = np.empty(N, np.int64)
    loc_of = np.empty(N, np.int64)
    core_of[order] = np.arange(N) % n_cores
    loc_of[order] = np.arange(N) // n_cores
    gl = core_of * nloc + loc_of
    padrow = [c * nloc + nloc - 1 for c in range(n_cores)]

    nat = np.full((n_cores, nloc), -1, np.int64)
    nat[core_of, loc_of] = np.arange(N)

    # hop order: per-core resort by hop degree desc (pads last)
    hkey = np.where(nat >= 0, -degh[np.maximum(nat, 0)], 1.0)
    hord = np.argsort(hkey, axis=1, kind="stable")
    hpos = np.argsort(hord, axis=1)

    selfg_pi = np.where(
        nat >= 0,
        np.arange(n_cores)[:, None] * nloc + np.arange(nloc)[None, :],
        np.array(padrow)[:, None],
    )
    K1, idx1 = _ell_build(
        gl[ei[0]], core_of[ei[1]], loc_of[ei[1]], None, n_cores, nloc, padrow
    )
    selfg_h = np.take_along_axis(selfg_pi, hord, axis=1)
    K3, idx3 = _ell_build(
        gl[eih[0]],
        core_of[eih[1]],
        hpos[core_of[eih[1]], loc_of[eih[1]]],
        selfg_h,
        n_cores,
        nloc,
        padrow,
    )

    in_maps = []
    for c in range(n_cores):
        natc = nat[c]
        real = natc >= 0
        xs = np.zeros((nloc, n_feat), np.float32)
        xs[real] = x[natc[real]]
        dis_c = np.ones(nloc, np.float32)
        dis_c[real] = dis[natc[real]]
        dish_pi = np.ones(nloc, np.float32)
        dish_pi[real] = dish[natc[real]]
        dishh = np.ones(nloc, np.float32)
        hnat = natc[hord[c]]
        hreal = hnat >= 0
        dishh[hreal] = dish[hnat[hreal]]
        gperm = np.full(nloc, padrow[c], np.int64)
        ratio = np.ones(nloc, np.float32)
        pv = perm[natc[real]]
        gperm[real] = gl[pv]
        ratio[real] = dis[natc[real]] / dis[pv]
        in_maps.append(
            {
                "xT": np.ascontiguousarray(xs.T),
                "dis_p": _plane(dis_c, nt),
                "dis2_p": _plane(dis_c * dis_c, nt),
                "dishp_p": _plane(dish_pi, nt),
                "dishh_p": _plane(dishh, nt),
                "ratio_p": _plane(ratio, nt),
                "gperm_p": _plane(gperm.astype(np.int32), nt),
                "idxR_p": _plane(hpos[c].astype(np.int32), nt),
                "idx1": idx1[c],
                "idx3": idx3[c],
                "W1": W1,
                "W2": W2,
                "W3": W3,
                "M1": M1,
                "M2": M2,
                "Wd0": Wd0,
                "Wc": np.ascontiguousarray(Wc),
            }
        )

    meta = dict(
        n_cores=n_cores,
        nloc=nloc,
        nt=nt,
        ng=ng,
        n_feat=n_feat,
        D=D,
        ncls=ncls,
        K1=K1,
        K3=K3,
        core_of=core_of,
        loc_of=loc_of,
    )
    return in_maps, meta


# ------------------------------------------------------------- device build


def build(meta):
    n_cores = meta["n_cores"]
    nloc, nt, ng = meta["nloc"], meta["nt"], meta["ng"]
    n_feat, D, ncls = meta["n_feat"], meta["D"], meta["ncls"]
    K1, K3 = meta["K1"], meta["K3"]
    DD = 2 * D
    nfc = n_feat // P
    sk1, sk3 = sum(K1), sum(K3)
    groups = [list(range(n_cores))]

    nc = bacc.Bacc("TRN2", debug=False, num_devices=n_cores)
    shared = "Shared" if n_cores > 4 else "Local"

    xT = nc.dram_tensor("xT", [n_feat, nloc], F32, kind="ExternalInput")
    dis_p = nc.dram_tensor("dis_p", [P, nt], F32, kind="ExternalInput")
    dis2_p = nc.dram_tensor("dis2_p", [P, nt], F32, kind="ExternalInput")
    dishp_p = nc.dram_tensor("dishp_p", [P, nt], F32, kind="ExternalInput")
    dishh_p = nc.dram_tensor("dishh_p", [P, nt], F32, kind="ExternalInput")
    ratio_p = nc.dram_tensor("ratio_p", [P, nt], F32, kind="ExternalInput")
    gperm_p = nc.dram_tensor("gperm_p", [P, nt], I32, kind="ExternalInput")
    idxR_p = nc.dram_tensor("idxR_p", [P, nt], I32, kind="ExternalInput")
    idx1 = nc.dram_tensor("idx1", [P, sk1], I32, kind="ExternalInput")
    idx3 = nc.dram_tensor("idx3", [P, sk3], I32, kind="ExternalInput")
    W1 = nc.dram_tensor("W1", [n_feat, D], F32, kind="ExternalInput")
    W2 = nc.dram_tensor("W2", [D, D], F32, kind="ExternalInput")
    W3 = nc.dram_tensor("W3", [D, D], F32, kind="ExternalInput")
    M1 = nc.dram_tensor("M1", [D, D], F32, kind="ExternalInput")
    M2 = nc.dram_tensor("M2", [D, D], F32, kind="ExternalInput")
    Wd0 = nc.dram_tensor("Wd0", [D, D], F32, kind="ExternalInput")
    Wc = nc.dram_tensor("Wc", [D, ncls], F32, kind="ExternalInput")
    out = nc.dram_tensor("out", [nloc, ncls + 2], F32, kind="ExternalOutput")

    xw_s = nc.dram_tensor("xw_s", [nloc, D], TDT, kind="Internal")
    XW = nc.dram_tensor("XW", [ng, D], TDT, kind="Internal", addr_space=shared)
    t1_s = nc.dram_tensor("t1_s", [nloc, DD], TDT, kind="Internal")
    T1 = nc.dram_tensor("T1", [ng, DD], TDT, kind="Internal", addr_space=shared)
    zd_s = nc.dram_tensor("zd_s", [nloc, DD], TDT, kind="Internal")
    ZD = nc.dram_tensor("ZD", [ng, DD], TDT, kind="Internal", addr_space=shared)
    e1_s = nc.dram_tensor("e1_s", [nloc, D], F32, kind="Internal")
    e1h_s = nc.dram_tensor("e1h_s", [nloc, DD], TDT, kind="Internal")
    e1d_s = nc.dram_tensor("e1d_s", [nloc, D], TDT, kind="Internal")
    E1H = nc.dram_tensor("E1H", [ng, DD], TDT, kind="Internal", addr_space=shared)
    E1D = nc.dram_tensor("E1D", [ng, D], TDT, kind="Internal", addr_space=shared)
    E2h = nc.dram_tensor("E2h", [nloc, DD], F32, kind="Internal")
    TV = nc.dram_tensor("TV", [nloc, D], F32, kind="Internal")

    with tile.TileContext(nc) as tc:
        with (
            tc.tile_pool(name="const", bufs=1) as constp,
            tc.tile_pool(name="gath", bufs=2) as gathp,
            tc.tile_pool(name="work", bufs=3) as workp,
            tc.tile_pool(name="outp", bufs=3) as outp,
            tc.tile_pool(name="psum", bufs=2, space="PSUM") as psp,
        ):
            ident = constp.tile([P, P], F32)
            make_identity(nc, ident[:])

            # resident planes + indices
            def res(t_dram, w, dt=F32, name=None):
                tl = constp.tile([P, w], dt, name=name)
                nc.sync.dma_start(tl[:], t_dram.ap())
                return tl

            disq = res(dis_p, nt, name="disq")
            dis2q = res(dis2_p, nt, name="dis2q")
            dishpq = res(dishp_p, nt, name="dishpq")
            dishhq = res(dishh_p, nt, name="dishhq")
            ratioq = res(ratio_p, nt, name="ratioq")
            gpermq = res(gperm_p, nt, I32, name="gpermq")
            idxRq = res(idxR_p, nt, I32, name="idxRq")
            idx1q = res(idx1, sk1, I32, name="idx1q")
            idx3q = res(idx3, sk3, I32, name="idx3q")

            w1t = [
                constp.tile([P, D], F32, name=f"w1t_{i}") for i in range(nfc)
            ]
            for i in range(nfc):
                nc.sync.dma_start(w1t[i][:], W1.ap()[i * P : (i + 1) * P])
            w2t = res(W2, D, name="w2t")
            w3t = res(W3, D, name="w3t")
            m1t = res(M1, D, name="m1t")
            m2t = res(M2, D, name="m2t")
            wdt = res(Wd0, D, name="wdt")
            wct = res(Wc, ncls, name="wct")

            def rows(t):
                return slice(t * P, (t + 1) * P)

            def col(plane, t):
                return plane[:, t : t + 1]

            # ---- S0: xW1' shard
            for t in range(nt):
                ps = psp.tile([P, D], F32, tag="mm")
                for i in range(nfc):
                    xt = workp.tile([P, P], F32, tag="xt")
                    nc.sync.dma_start(xt[:], xT.ap()[i * P : (i + 1) * P, rows(t)])
                    nc.tensor.matmul(
                        out=ps[:],
                        lhsT=xt[:],
                        rhs=w1t[i][:],
                        start=(i == 0),
                        stop=(i == nfc - 1),
                    )
                o = outp.tile([P, D], TDT, tag="s0")
                nc.vector.tensor_scalar_mul(o[:], ps[:], col(disq, t))
                nc.sync.dma_start(xw_s.ap()[rows(t)], o[:])

            nc.gpsimd.collective_compute(
                "AllGather",
                ALU.bypass,
                replica_groups=groups,
                ins=[xw_s[:].opt()],
                outs=[XW[:].opt()],
            )

            # ---- S2: T1 shard
            for t in range(nt):
                g = gathp.tile([P, D], TDT, tag="g2")
                nc.gpsimd.indirect_dma_start(
                    out=g[:],
                    out_offset=None,
                    in_=XW.ap(),
                    in_offset=bass.IndirectOffsetOnAxis(ap=col(gpermq, t), axis=0),
                )
                o = outp.tile([P, D], TDT, tag="s2")
                nc.vector.tensor_scalar_mul(o[:], g[:], col(ratioq, t))
                nc.sync.dma_start(t1_s.ap()[rows(t), D:DD], o[:])
                l = workp.tile([P, D], TDT, tag="s2l")
                nc.sync.dma_start(l[:], xw_s.ap()[rows(t)])
                nc.sync.dma_start(t1_s.ap()[rows(t), 0:D], l[:])

            nc.gpsimd.collective_compute(
                "AllGather",
                ALU.bypass,
                replica_groups=groups,
                ins=[t1_s[:].opt()],
                outs=[T1[:].opt()],
            )

            # ---- per-slot indirect ELL gather driver -------------------
            # one indirect DMA per (tile, slot): [P,1] offset column gathers
            # 128 rows; slots accumulate via a vector reduce. The self-loop
            # term is a local-tile add (local_s) instead of an ELL slot.
            def ell_run(table, width, Ks, idxq, local_s, tail):
                koff = 0
                for t in range(nt):
                    K = Ks[t]
                    g = gathp.tile([P, K * width], TDT, tag="ge")
                    for k in range(K):
                        nc.gpsimd.indirect_dma_start(
                            out=g[:, k * width : (k + 1) * width],
                            out_offset=None,
                            in_=table.ap(),
                            in_offset=bass.IndirectOffsetOnAxis(
                                ap=idxq[:, koff + k : koff + k + 1], axis=0
                            ),
                        )
                    koff += K
                    s = workp.tile([P, width], F32, tag="se")
                    if K == 1:
                        nc.vector.tensor_copy(s[:], g[:])
                    else:
                        nc.vector.tensor_reduce(
                            out=s[:],
                            in_=g[:].rearrange("p (k d) -> p d k", k=K),
                            axis=mybir.AxisListType.X,
                            op=ALU.add,
                        )
                    if local_s is not None:
                        li = workp.tile([P, width], TDT, tag="sl")
                        nc.sync.dma_start(li[:], local_s.ap()[rows(t)])
                        nc.vector.tensor_tensor(
                            out=s[:], in0=s[:], in1=li[:], op=ALU.add
                        )
                    tail(t, s)

            # ---- G1: zd = relu(dis2 * sum) -> zd_s
            def g1_tail(t, s):
                o = outp.tile([P, DD], TDT, tag="ze")
                nc.vector.tensor_scalar(
                    o[:], s[:], col(dis2q, t), 0.0, ALU.mult, ALU.max
                )
                nc.sync.dma_start(zd_s.ap()[rows(t)], o[:])

            ell_run(T1, DD, K1, idx1q, t1_s, g1_tail)

            nc.gpsimd.collective_compute(
                "AllGather",
                ALU.bypass,
                replica_groups=groups,
                ins=[zd_s[:].opt()],
                outs=[ZD[:].opt()],
            )

            # ---- G2: S @ W2, three shipped variants
            def g2_tail(t, s):
                e1h = outp.tile([P, DD], TDT, tag="e1h")
                e1d = outp.tile([P, D], TDT, tag="e1d")
                e1p = outp.tile([P, D], F32, tag="e1p")
                for h in range(2):
                    tp = psp.tile([P, P], F32, tag="t", bufs=3)
                    nc.tensor.transpose(
                        out=tp[:], in_=s[:, h * D : (h + 1) * D], identity=ident[:]
                    )
                    tps = workp.tile([P, P], F32, tag="tps")
                    nc.vector.tensor_copy(tps[:], tp[:])
                    mm = psp.tile([P, D], F32, tag="m", bufs=3)
                    nc.tensor.matmul(
                        out=mm[:], lhsT=tps[:], rhs=w2t[:], start=True, stop=True
                    )
                    # e1 = relu(dis * mm)
                    eh = workp.tile([P, D], F32, tag="eh")
                    nc.vector.tensor_scalar(
                        eh[:], mm[:], col(disq, t), 0.0, ALU.mult, ALU.max
                    )
                    nc.vector.tensor_scalar_mul(
                        e1h[:, h * D : (h + 1) * D], eh[:], col(dishpq, t)
                    )
                    if h == 0:
                        nc.vector.tensor_copy(e1p[:], eh[:])
                        nc.vector.tensor_scalar_mul(e1d[:], eh[:], col(disq, t))
                nc.sync.dma_start(e1_s.ap()[rows(t)], e1p[:])
                nc.sync.dma_start(e1h_s.ap()[rows(t)], e1h[:])
                nc.sync.dma_start(e1d_s.ap()[rows(t)], e1d[:])

            ell_run(ZD, DD, K1, idx1q, zd_s, g2_tail)

            nc.gpsimd.collective_compute(
                "AllGather",
                ALU.bypass,
                replica_groups=groups,
                ins=[e1h_s[:].opt()],
                outs=[E1H[:].opt()],
            )
            nc.gpsimd.collective_compute(
                "AllGather",
                ALU.bypass,
                replica_groups=groups,
                ins=[e1d_s[:].opt()],
                outs=[E1D[:].opt()],
            )

            # ---- S12: MLP + tvec (local, overlaps with AG3/G3)
            for t in range(nt):
                et = workp.tile([P, D], F32, tag="ml_in")
                nc.sync.dma_start(et[:], e1_s.ap()[rows(t)])
                tp = psp.tile([P, P], F32, tag="t", bufs=3)
                nc.tensor.transpose(out=tp[:], in_=et[:], identity=ident[:])
                tps = workp.tile([P, P], F32, tag="tps")
                nc.vector.tensor_copy(tps[:], tp[:])
                mm = psp.tile([P, D], F32, tag="m", bufs=3)
                nc.tensor.matmul(out=mm[:], lhsT=tps[:], rhs=m1t[:], start=True, stop=True)
                u = workp.tile([P, D], F32, tag="ml_u")
                nc.scalar.activation(u[:], mm[:], AF.Relu)
                tp2 = psp.tile([P, P], F32, tag="t", bufs=3)
                nc.tensor.transpose(out=tp2[:], in_=u[:], identity=ident[:])
                tps2 = workp.tile([P, P], F32, tag="tps")
                nc.vector.tensor_copy(tps2[:], tp2[:])
                mm2 = psp.tile([P, D], F32, tag="m", bufs=3)
                nc.tensor.matmul(
                    out=mm2[:], lhsT=tps2[:], rhs=m2t[:], start=True, stop=True
                )
                e3 = workp.tile([P, D], F32, tag="ml_e3")
                nc.vector.tensor_copy(e3[:], mm2[:])
                tp3 = psp.tile([P, P], F32, tag="t", bufs=3)
                nc.tensor.transpose(out=tp3[:], in_=e3[:], identity=ident[:])
                tps3 = workp.tile([P, P], F32, tag="tps")
                nc.vector.tensor_copy(tps3[:], tp3[:])
                mm3 = psp.tile([P, D], F32, tag="m", bufs=3)
                nc.tensor.matmul(
                    out=mm3[:], lhsT=tps3[:], rhs=wdt[:], start=True, stop=True
                )
                tv = outp.tile([P, D], F32, tag="ml_tv")
                nc.vector.tensor_copy(tv[:], mm3[:])
                nc.sync.dma_start(TV.ap()[rows(t)], tv[:])

            # ---- G3: embed2{,b} = dishh * (S_h @ W3) -> E2h (hop order)
            def g3_tail(t, s):
                e2 = outp.tile([P, DD], F32, tag="e2")
                for h in range(2):
                    tp = psp.tile([P, P], F32, tag="t", bufs=3)
                    nc.tensor.transpose(
                        out=tp[:], in_=s[:, h * D : (h + 1) * D], identity=ident[:]
                    )
                    tps = workp.tile([P, P], F32, tag="tps")
                    nc.vector.tensor_copy(tps[:], tp[:])
                    mm = psp.tile([P, D], F32, tag="m", bufs=3)
                    nc.tensor.matmul(
                        out=mm[:], lhsT=tps[:], rhs=w3t[:], start=True, stop=True
                    )
                    nc.vector.tensor_scalar_mul(
                        e2[:, h * D : (h + 1) * D], mm[:], col(dishhq, t)
                    )
                nc.sync.dma_start(E2h.ap()[rows(t)], e2[:])

            ell_run(E1H, DD, K3, idx3q, None, g3_tail)

            # ---- S11 + S13: realign + scores
            for t in range(nt):
                e2 = gathp.tile([P, DD], F32, tag="gr")
                nc.gpsimd.indirect_dma_start(
                    out=e2[:],
                    out_offset=None,
                    in_=E2h.ap(),
                    in_offset=bass.IndirectOffsetOnAxis(ap=col(idxRq, t), axis=0),
                )
                tv = workp.tile([P, D], F32, tag="sc_tv")
                nc.sync.dma_start(tv[:], TV.ap()[rows(t)])
                pr = workp.tile([P, DD], F32, tag="sc_pr")
                nc.vector.tensor_mul(pr[:, 0:D], tv[:], e2[:, 0:D])
                nc.vector.tensor_mul(pr[:, D:DD], tv[:], e2[:, D:DD])
                rs = workp.tile([P, 2], F32, tag="sc_rs")
                nc.vector.tensor_reduce(
                    out=rs[:],
                    in_=pr[:].rearrange("p (h d) -> p h d", h=2),
                    axis=mybir.AxisListType.X,
                    op=ALU.add,
                )
                sg = outp.tile([P, 2], F32, tag="sc_sg")
                nc.scalar.activation(sg[:], rs[:], AF.Sigmoid)
                nc.sync.dma_start(out.ap()[rows(t), ncls : ncls + 2], sg[:])

            # ---- G4: cls = (dis * sum) @ Wc -> out[:, :ncls]
            def g4_tail(t, s):
                sc_ = workp.tile([P, D], F32, tag="c_s")
                nc.vector.tensor_scalar_mul(sc_[:], s[:], col(disq, t))
                tp = psp.tile([P, P], F32, tag="t", bufs=3)
                nc.tensor.transpose(out=tp[:], in_=sc_[:], identity=ident[:])
                tps = workp.tile([P, P], F32, tag="tps")
                nc.vector.tensor_copy(tps[:], tp[:])
                mm = psp.tile([P, ncls], F32, tag="m", bufs=3)
                nc.tensor.matmul(out=mm[:], lhsT=tps[:], rhs=wct[:], start=True, stop=True)
                o = outp.tile([P, ncls], F32, tag="c_o")
                nc.vector.tensor_copy(o[:], mm[:])
                nc.sync.dma_start(out.ap()[rows(t), 0:ncls], o[:])

            ell_run(E1D, D, K1, idx1q, e1d_s, g4_tail)

    nc.compile()
    return nc


def assemble(results, meta):
    n_cores = meta["n_cores"]
    N = len(meta["core_of"])
    ncls = meta["ncls"]
    out = np.empty((N, ncls + 2), np.float32)
    for c in range(n_cores):
        oc = results[c]["out"]
        m = meta["core_of"] == c
        out[m] = oc[meta["loc_of"][m]]
    return out


# ------------------------------------------------------------------ entry


_CACHE = {}
TRACE = False
LAST_RES = None


def kernel(**inputs):
    """Full-input entry point: shards across 8 NeuronCores internally.

    Expects the nn_MixModel input dict (x, edge_index, edge_index_hop, y,
    perm, W1..Wd); returns the full [N, n_cls+2] float32 output.
    """
    n_cores = 8
    in_maps, meta = prep(inputs, n_cores)
    key = (meta["nloc"], tuple(meta["K1"]), tuple(meta["K3"]))
    nc = _CACHE.get(key)
    if nc is None:
        nc = build(meta)
        _CACHE[key] = nc
    res = bass_utils.run_bass_kernel_spmd(
        nc, in_maps, core_ids=list(range(n_cores)), trace=TRACE
    )
    global LAST_RES
    LAST_RES = res
    return assemble(res.results, meta)


# revision 11
# speedup vs baseline: 1.8807x; 1.0418x over previous
"""8-core Trainium2 Bass kernel for nn_MixModel (GCN mix model) — v3.

Sharding: nodes dealt round-robin by in-degree rank to 8 cores; each core owns
NLOC = ceil((ceil(N/8)+1)/128)*128 local rows (>=1 zero pad row reused as the
ELL gather-pad target).

Algebra used:
 - GCN messages factorize: msg = (h*dis)[src], output scaled by dis[dst]; the
   self-loop term is a local-tile add (pi-order stages) or an extra ELL slot
   (hop stage). Aggregation = unweighted padded-ELL gather+sum of pre-scaled
   table rows.
 - segsum and the layer matmul commute: sum((z@W*dis)[src]) =
   sum((z*dis)[src]) @ W — so cores AllGather the *scaled activations* and the
   per-layer matmul runs on the 98 aggregated dst tiles.
 - good/bad paths share edge sets -> gather concatenated 256-wide tables.
 - the permuted-input path's first-layer table is a cheap local permutation
   gather of the xW1' table (12.5k rows), not a per-edge pass.

Gather engine: gpsimd.dma_gather (InstDMAGatherAnt, mlp ucode library) with
int16 indices. The 100352-row shared tables exceed int16 range, so each ELL
tile is split into 4 chunk rectangles (chunk = 25088 consecutive table rows =
one core pair); chunk-local indices fit int16. Rectangles of consecutive tiles
are batched into one dma_gather call per chunk (<=32 j-columns per call,
<=96 per batch) amortizing the ~1us SWDGE fixed overhead over thousands of
row descriptors. Pad slots point at the chunk's zero pad row (local nloc-1).

Stages (per core):
  S0   xW1' shard = (x_sh @ W1) * dis_sh
  AG0  AllGather -> XW [NG,128]
  S2   T1 shard = [xW1'_loc | gather(XW, gperm)*ratio] ; AG1 -> T1 [NG,256]
  G1   ELL gather T1 -> zd = relu(dis^2 * sum)  (= z1*dis)      -> AG2 ZD
  G2   ELL gather ZD -> S ; e1{,b} = relu(dis * (S_h @ W2)) ;
       ship [e1*dish|e1b*dish] -> AG3a E1H ; [e1*dis] -> AG3b E1D ; e1 local
  G3   ELL gather E1H (hop order) -> embed2{,b} = dish * (S_h @ W3) -> E2h
  S12  MLP: embed3 = relu(e1@M1)@M2 ; tvec = embed3@Wd0
  S11  realign E2h to pi order ; scores = sigmoid(rowsum(tvec * e2{,b}))
  G4   ELL gather E1D -> cls = (dis*sum)@Wc -> OUT[:, :10]
"""

import numpy as np

import concourse.bacc as bacc
import concourse.bass as bass
import concourse.mybir as mybir
import concourse.tile as tile
from concourse import bass_utils
from concourse.masks import make_identity

P = 128
F32 = mybir.dt.float32
I32 = mybir.dt.int32
I16 = mybir.dt.int16
AF = mybir.ActivationFunctionType
ALU = mybir.AluOpType
TDT = mybir.dt.bfloat16  # transport/table dtype

# ----------------------------------------------------------------- host prep


def _ell_build(src_g, dst_core, dst_loc, self_g, n_cores, nloc, padrow):
    """Shared-K ELL: returns (K per tile, per-core int32 [P, sum(K)] arrays,
    p-major-global: element [p, koff[t]+k] = slot k of local row t*128+p)."""
    nt = nloc // P
    counts = np.zeros((n_cores, nloc), np.int64)
    np.add.at(counts, (dst_core, dst_loc), 1)
    n_self = 0 if self_g is None else 1
    cmax = counts.reshape(n_cores, nt, P).max(axis=(0, 2))
    K = (cmax + n_self).astype(np.int64)
    order = np.lexsort((dst_loc, dst_core))
    sc, sl, sg = dst_core[order], dst_loc[order], src_g[order]
    key = sc.astype(np.int64) * nloc + sl
    is_start = np.r_[True, key[1:] != key[:-1]] if len(key) else np.array([], bool)
    run_starts = np.flatnonzero(is_start)
    run_len = np.diff(np.r_[run_starts, len(key)])
    pos_in_run = np.arange(len(key)) - np.repeat(run_starts, run_len)
    koff = np.r_[0, np.cumsum(K)]
    sk = int(koff[-1])
    idx_arrs = []
    for c in range(n_cores):
        arr = np.full((P, sk), padrow[c], np.int64)
        m = sc == c
        loc, pos, gidx = sl[m], pos_in_run[m], sg[m]
        t = loc // P
        p = loc % P
        arr[p, koff[t] + pos + n_self] = gidx
        if n_self:
            allt = np.arange(nloc) // P
            allp = np.arange(nloc) % P
            arr[allp, koff[allt]] = self_g[c]
        idx_arrs.append(arr.astype(np.int32))
    return K.tolist(), idx_arrs


def _plane(vals_loc, nt):
    """[nloc] local-row vector -> [P, nt] plane (local row t*128+p -> [p, t])."""
    return np.ascontiguousarray(vals_loc.reshape(nt, P).T)


def prep(inputs, n_cores=8):
    x = np.asarray(inputs["x"], np.float32)
    ei = np.asarray(inputs["edge_index"], np.int64)
    eih = np.asarray(inputs["edge_index_hop"], np.int64)
    perm = np.asarray(inputs["perm"], np.int64)
    W1 = np.asarray(inputs["W1"], np.float32)
    W2 = np.asarray(inputs["W2"], np.float32)
    W3 = np.asarray(inputs["W3"], np.float32)
    M1 = np.asarray(inputs["M1"], np.float32)
    M2 = np.asarray(inputs["M2"], np.float32)
    Wc = np.asarray(inputs["Wc"], np.float32)
    Wd0 = np.asarray(inputs["Wd"], np.float32)[0]
    for bname in ("b1", "b2", "b3", "mb1", "mb2", "bc"):
        assert np.abs(np.asarray(inputs[bname])).max() == 0.0, (
            f"nonzero bias {bname} not supported by this kernel build"
        )

    N, n_feat = x.shape
    D = W1.shape[1]
    ncls = Wc.shape[1]
    max_real = -(-N // n_cores)
    nloc = -(-(max_real + 1) // P) * P
    nt = nloc // P
    ng = n_cores * nloc

    deg = np.bincount(ei[1], minlength=N).astype(np.float32) + 1.0
    degh = np.bincount(eih[1], minlength=N).astype(np.float32) + 1.0
    dis = 1.0 / np.sqrt(deg)
    dish = 1.0 / np.sqrt(degh)

    order = np.argsort(-deg, kind="stable")
    core_of = np.empty(N, np.int64)
    loc_of = np.empty(N, np.int64)
    core_of[order] = np.arange(N) % n_cores
    loc_of[order] = np.arange(N) // n_cores

    # piece-major global table layout: [piece][core][piece-local rows], so
    # each split-AllGather piece lands contiguously in the shared tables.
    NSPL = 4
    pr0 = np.array([(nt * i // NSPL) * P for i in range(NSPL + 1)], np.int64)

    def glmap(c, r):
        p = np.searchsorted(pr0, r, side="right") - 1
        return 8 * pr0[p] + c * (pr0[p + 1] - pr0[p]) + (r - pr0[p])

    gl = glmap(core_of, loc_of)
    padrow = [int(glmap(np.int64(c), np.int64(nloc - 1))) for c in range(n_cores)]

    nat = np.full((n_cores, nloc), -1, np.int64)
    nat[core_of, loc_of] = np.arange(N)

    # hop order: per-core resort by hop degree desc (pads last)
    hkey = np.where(nat >= 0, -degh[np.maximum(nat, 0)], 1.0)
    hord = np.argsort(hkey, axis=1, kind="stable")
    hpos = np.argsort(hord, axis=1)

    allg = glmap(
        np.repeat(np.arange(n_cores), nloc), np.tile(np.arange(nloc), n_cores)
    ).reshape(n_cores, nloc)
    selfg_pi = np.where(nat >= 0, allg, np.array(padrow)[:, None])
    K1, idx1 = _ell_build(
        gl[ei[0]], core_of[ei[1]], loc_of[ei[1]], None, n_cores, nloc, padrow
    )
    selfg_h = np.take_along_axis(selfg_pi, hord, axis=1)
    K3, idx3 = _ell_build(
        gl[eih[0]],
        core_of[eih[1]],
        hpos[core_of[eih[1]], loc_of[eih[1]]],
        selfg_h,
        n_cores,
        nloc,
        padrow,
    )

    in_maps = []
    for c in range(n_cores):
        natc = nat[c]
        real = natc >= 0
        xs = np.zeros((nloc, n_feat), np.float32)
        xs[real] = x[natc[real]]
        dis_c = np.ones(nloc, np.float32)
        dis_c[real] = dis[natc[real]]
        dish_pi = np.ones(nloc, np.float32)
        dish_pi[real] = dish[natc[real]]
        dishh = np.ones(nloc, np.float32)
        hnat = natc[hord[c]]
        hreal = hnat >= 0
        dishh[hreal] = dish[hnat[hreal]]
        gperm = np.full(nloc, padrow[c], np.int64)
        ratio = np.ones(nloc, np.float32)
        pv = perm[natc[real]]
        gperm[real] = gl[pv]
        ratio[real] = dis[natc[real]] / dis[pv]
        in_maps.append(
            {
                "xTb": np.ascontiguousarray(
                    xs.reshape(nt, P, 4, P).transpose(0, 2, 3, 1).reshape(nt * 4 * P, P)
                ),
                "dis_p": _plane(dis_c, nt),
                "dis2_p": _plane(dis_c * dis_c, nt),
                "dishp_p": _plane(dish_pi, nt),
                "dishh_p": _plane(dishh, nt),
                "ratio_p": _plane(ratio, nt),
                "gperm_p": _plane(gperm.astype(np.int32), nt),
                "idxR_p": _plane(hpos[c].astype(np.int32), nt),
                "idx1": idx1[c],
                "idx3": idx3[c],
                "W1": W1,
                "W2": W2,
                "W3": W3,
                "M1": M1,
                "M2": M2,
                "Wd0": Wd0,
                "Wc": np.ascontiguousarray(Wc),
            }
        )

    meta = dict(
        n_cores=n_cores,
        nloc=nloc,
        nt=nt,
        ng=ng,
        n_feat=n_feat,
        D=D,
        ncls=ncls,
        K1=K1,
        K3=K3,
        core_of=core_of,
        loc_of=loc_of,
    )
    return in_maps, meta


# ------------------------------------------------------------- device build


def build(meta):
    n_cores = meta["n_cores"]
    nloc, nt, ng = meta["nloc"], meta["nt"], meta["ng"]
    n_feat, D, ncls = meta["n_feat"], meta["D"], meta["ncls"]
    K1, K3 = meta["K1"], meta["K3"]
    DD = 2 * D
    nfc = n_feat // P
    sk1, sk3 = sum(K1), sum(K3)
    groups = [list(range(n_cores))]

    nc = bacc.Bacc("TRN2", debug=False, num_devices=n_cores)
    shared = "Shared" if n_cores > 4 else "Local"

    xTb = nc.dram_tensor("xTb", [nt * 4 * P, P], F32, kind="ExternalInput")
    dis_p = nc.dram_tensor("dis_p", [P, nt], F32, kind="ExternalInput")
    dis2_p = nc.dram_tensor("dis2_p", [P, nt], F32, kind="ExternalInput")
    dishp_p = nc.dram_tensor("dishp_p", [P, nt], F32, kind="ExternalInput")
    dishh_p = nc.dram_tensor("dishh_p", [P, nt], F32, kind="ExternalInput")
    ratio_p = nc.dram_tensor("ratio_p", [P, nt], F32, kind="ExternalInput")
    gperm_p = nc.dram_tensor("gperm_p", [P, nt], I32, kind="ExternalInput")
    idxR_p = nc.dram_tensor("idxR_p", [P, nt], I32, kind="ExternalInput")
    idx1 = nc.dram_tensor("idx1", [P, sk1], I32, kind="ExternalInput")
    idx3 = nc.dram_tensor("idx3", [P, sk3], I32, kind="ExternalInput")
    W1 = nc.dram_tensor("W1", [n_feat, D], F32, kind="ExternalInput")
    W2 = nc.dram_tensor("W2", [D, D], F32, kind="ExternalInput")
    W3 = nc.dram_tensor("W3", [D, D], F32, kind="ExternalInput")
    M1 = nc.dram_tensor("M1", [D, D], F32, kind="ExternalInput")
    M2 = nc.dram_tensor("M2", [D, D], F32, kind="ExternalInput")
    Wd0 = nc.dram_tensor("Wd0", [D, D], F32, kind="ExternalInput")
    Wc = nc.dram_tensor("Wc", [D, ncls], F32, kind="ExternalInput")
    out = nc.dram_tensor("out", [nloc, ncls + 2], F32, kind="ExternalOutput")

    xw_s = nc.dram_tensor("xw_s", [nloc, D], TDT, kind="Internal")
    XW = nc.dram_tensor("XW", [ng, D], TDT, kind="Internal", addr_space=shared)
    t1_s = nc.dram_tensor("t1_s", [nloc, DD], TDT, kind="Internal")
    T1 = nc.dram_tensor("T1", [ng, DD], TDT, kind="Internal", addr_space=shared)
    zd_s = nc.dram_tensor("zd_s", [nloc, DD], TDT, kind="Internal")
    ZD = nc.dram_tensor("ZD", [ng, DD], TDT, kind="Internal", addr_space=shared)
    e1_s = nc.dram_tensor("e1_s", [nloc, D], F32, kind="Internal")
    e1h_s = nc.dram_tensor("e1h_s", [nloc, DD], TDT, kind="Internal")
    e1d_s = nc.dram_tensor("e1d_s", [nloc, D], TDT, kind="Internal")
    E1H = nc.dram_tensor("E1H", [ng, DD], TDT, kind="Internal", addr_space=shared)
    E1D = nc.dram_tensor("E1D", [ng, D], TDT, kind="Internal", addr_space=shared)
    E2h = nc.dram_tensor("E2h", [nloc, DD], F32, kind="Internal")
    TV = nc.dram_tensor("TV", [nloc, D], F32, kind="Internal")

    with tile.TileContext(nc) as tc:
        with (
            tc.tile_pool(name="const", bufs=1) as constp,
            tc.tile_pool(name="gath", bufs=4) as gathp,
            tc.tile_pool(name="work", bufs=3) as workp,
            tc.tile_pool(name="outp", bufs=3) as outp,
            tc.tile_pool(name="psum", bufs=2, space="PSUM") as psp,
        ):
            ident = constp.tile([P, P], F32)
            make_identity(nc, ident[:])

            # resident planes + indices
            def res(t_dram, w, dt=F32, name=None):
                tl = constp.tile([P, w], dt, name=name)
                nc.sync.dma_start(tl[:], t_dram.ap())
                return tl

            disq = res(dis_p, nt, name="disq")
            dis2q = res(dis2_p, nt, name="dis2q")
            dishpq = res(dishp_p, nt, name="dishpq")
            dishhq = res(dishh_p, nt, name="dishhq")
            ratioq = res(ratio_p, nt, name="ratioq")
            gpermq = res(gperm_p, nt, I32, name="gpermq")
            idxRq = res(idxR_p, nt, I32, name="idxRq")
            idx1q = res(idx1, sk1, I32, name="idx1q")
            idx3q = res(idx3, sk3, I32, name="idx3q")

            w1t = [
                constp.tile([P, D], F32, name=f"w1t_{i}") for i in range(nfc)
            ]
            for i in range(nfc):
                nc.sync.dma_start(w1t[i][:], W1.ap()[i * P : (i + 1) * P])
            w2t = res(W2, D, name="w2t")
            w3t = res(W3, D, name="w3t")
            m1t = res(M1, D, name="m1t")
            m2t = res(M2, D, name="m2t")
            wdt = res(Wd0, D, name="wdt")
            wct = res(Wc, ncls, name="wct")

            def rows(t):
                return slice(t * P, (t + 1) * P)

            def col(plane, t):
                return plane[:, t : t + 1]

            NSPL = 4
            bound = [nt * (i + 1) // NSPL - 1 for i in range(NSPL)]

            def ag_piece(src, dst, piece):
                r0 = (nt * piece // NSPL) * P
                r1 = (nt * (piece + 1) // NSPL) * P
                nc.gpsimd.collective_compute(
                    "AllGather",
                    ALU.bypass,
                    replica_groups=groups,
                    ins=[src[r0:r1].opt()],
                    outs=[dst[n_cores * r0 : n_cores * r1].opt()],
                )

            # ---- S0: xW1' shard
            sp = 0
            for t in range(nt):
                ps = psp.tile([P, D], F32, tag="mm")
                for i in range(nfc):
                    xt = workp.tile([P, P], F32, tag="xt")
                    nc.sync.dma_start(
                        xt[:], xTb.ap()[(t * nfc + i) * P : (t * nfc + i + 1) * P]
                    )
                    nc.tensor.matmul(
                        out=ps[:],
                        lhsT=xt[:],
                        rhs=w1t[i][:],
                        start=(i == 0),
                        stop=(i == nfc - 1),
                    )
                o = outp.tile([P, D], TDT, tag="s0")
                nc.vector.tensor_scalar_mul(o[:], ps[:], col(disq, t))
                nc.sync.dma_start(xw_s.ap()[rows(t)], o[:])
                nc.sync.dma_start(t1_s.ap()[rows(t), 0:D], o[:])
                if t == bound[sp]:
                    ag_piece(xw_s, XW, sp)
                    sp += 1

            # ---- S2: T1 shard (bad half; good half written by S0)
            sp = 0
            for t in range(nt):
                g = gathp.tile([P, D], TDT, tag="g2")
                nc.gpsimd.indirect_dma_start(
                    out=g[:],
                    out_offset=None,
                    in_=XW.ap(),
                    in_offset=bass.IndirectOffsetOnAxis(ap=col(gpermq, t), axis=0),
                )
                o = outp.tile([P, D], TDT, tag="s2")
                nc.vector.tensor_scalar_mul(o[:], g[:], col(ratioq, t))
                nc.sync.dma_start(t1_s.ap()[rows(t), D:DD], o[:])
                if t == bound[sp]:
                    ag_piece(t1_s, T1, sp)
                    sp += 1

            # ---- per-slot indirect ELL gather driver -------------------
            # one indirect DMA per (tile, slot): [P,1] offset column gathers
            # 128 rows; slots accumulate via a vector reduce. The self-loop
            # term is a local-tile add (local_s) instead of an ELL slot.
            def ell_run(table, width, Ks, idxq, local_s, tail):
                koff = 0
                for t in range(nt):
                    K = Ks[t]
                    g = gathp.tile([P, K * width], TDT, tag="ge")
                    for k in range(K):
                        nc.gpsimd.indirect_dma_start(
                            out=g[:, k * width : (k + 1) * width],
                            out_offset=None,
                            in_=table.ap(),
                            in_offset=bass.IndirectOffsetOnAxis(
                                ap=idxq[:, koff + k : koff + k + 1], axis=0
                            ),
                        )
                    koff += K
                    s = workp.tile([P, width], F32, tag="se")
                    if K == 1:
                        nc.vector.tensor_copy(s[:], g[:])
                    else:
                        nc.vector.tensor_reduce(
                            out=s[:],
                            in_=g[:].rearrange("p (k d) -> p d k", k=K),
                            axis=mybir.AxisListType.X,
                            op=ALU.add,
                        )
                    if local_s is not None:
                        li = workp.tile([P, width], TDT, tag="sl")
                        nc.sync.dma_start(li[:], local_s.ap()[rows(t)])
                        nc.vector.tensor_tensor(
                            out=s[:], in0=s[:], in1=li[:], op=ALU.add
                        )
                    tail(t, s)

            # ---- G1: zd = relu(dis2 * sum) -> zd_s
            spl = [0]

            def g1_tail(t, s):
                o = outp.tile([P, DD], TDT, tag="ze")
                nc.vector.tensor_scalar(
                    o[:], s[:], col(dis2q, t), 0.0, ALU.mult, ALU.max
                )
                nc.sync.dma_start(zd_s.ap()[rows(t)], o[:])
                if t == bound[spl[0]]:
                    ag_piece(zd_s, ZD, spl[0])
                    spl[0] += 1

            ell_run(T1, DD, K1, idx1q, t1_s, g1_tail)

            # ---- G2: S @ W2, three shipped variants
            def g2_tail(t, s):
                e1h = outp.tile([P, DD], TDT, tag="e1h")
                e1d = outp.tile([P, D], TDT, tag="e1d")
                e1p = outp.tile([P, D], F32, tag="e1p")
                for h in range(2):
                    tp = psp.tile([P, P], F32, tag="t", bufs=3)
                    nc.tensor.transpose(
                        out=tp[:], in_=s[:, h * D : (h + 1) * D], identity=ident[:]
                    )
                    tps = workp.tile([P, P], F32, tag="tps")
                    nc.vector.tensor_copy(tps[:], tp[:])
                    mm = psp.tile([P, D], F32, tag="m", bufs=3)
                    nc.tensor.matmul(
                        out=mm[:], lhsT=tps[:], rhs=w2t[:], start=True, stop=True
                    )
                    # e1 = relu(dis * mm)
                    eh = workp.tile([P, D], F32, tag="eh")
                    nc.vector.tensor_scalar(
                        eh[:], mm[:], col(disq, t), 0.0, ALU.mult, ALU.max
                    )
                    nc.vector.tensor_scalar_mul(
                        e1h[:, h * D : (h + 1) * D], eh[:], col(dishpq, t)
                    )
                    if h == 0:
                        nc.vector.tensor_copy(e1p[:], eh[:])
                        nc.vector.tensor_scalar_mul(e1d[:], eh[:], col(disq, t))
                nc.sync.dma_start(e1_s.ap()[rows(t)], e1p[:])
                nc.sync.dma_start(e1h_s.ap()[rows(t)], e1h[:])
                nc.sync.dma_start(e1d_s.ap()[rows(t)], e1d[:])
                if t == bound[spl[0]]:
                    ag_piece(e1h_s, E1H, spl[0])
                    ag_piece(e1d_s, E1D, spl[0])
                    spl[0] += 1

            spl[0] = 0
            ell_run(ZD, DD, K1, idx1q, zd_s, g2_tail)

            # ---- S12: MLP + tvec (local, overlaps with AG3/G3)
            for t in range(nt):
                et = workp.tile([P, D], F32, tag="ml_in")
                nc.sync.dma_start(et[:], e1_s.ap()[rows(t)])
                tp = psp.tile([P, P], F32, tag="t", bufs=3)
                nc.tensor.transpose(out=tp[:], in_=et[:], identity=ident[:])
                tps = workp.tile([P, P], F32, tag="tps")
                nc.vector.tensor_copy(tps[:], tp[:])
                mm = psp.tile([P, D], F32, tag="m", bufs=3)
                nc.tensor.matmul(out=mm[:], lhsT=tps[:], rhs=m1t[:], start=True, stop=True)
                u = workp.tile([P, D], F32, tag="ml_u")
                nc.scalar.activation(u[:], mm[:], AF.Relu)
                tp2 = psp.tile([P, P], F32, tag="t", bufs=3)
                nc.tensor.transpose(out=tp2[:], in_=u[:], identity=ident[:])
                tps2 = workp.tile([P, P], F32, tag="tps")
                nc.vector.tensor_copy(tps2[:], tp2[:])
                mm2 = psp.tile([P, D], F32, tag="m", bufs=3)
                nc.tensor.matmul(
                    out=mm2[:], lhsT=tps2[:], rhs=m2t[:], start=True, stop=True
                )
                e3 = workp.tile([P, D], F32, tag="ml_e3")
                nc.vector.tensor_copy(e3[:], mm2[:])
                tp3 = psp.tile([P, P], F32, tag="t", bufs=3)
                nc.tensor.transpose(out=tp3[:], in_=e3[:], identity=ident[:])
                tps3 = workp.tile([P, P], F32, tag="tps")
                nc.vector.tensor_copy(tps3[:], tp3[:])
                mm3 = psp.tile([P, D], F32, tag="m", bufs=3)
                nc.tensor.matmul(
                    out=mm3[:], lhsT=tps3[:], rhs=wdt[:], start=True, stop=True
                )
                tv = outp.tile([P, D], F32, tag="ml_tv")
                nc.vector.tensor_copy(tv[:], mm3[:])
                nc.sync.dma_start(TV.ap()[rows(t)], tv[:])

            # ---- G3: embed2{,b} = dishh * (S_h @ W3) -> E2h (hop order)
            def g3_tail(t, s):
                e2 = outp.tile([P, DD], F32, tag="e2")
                for h in range(2):
                    tp = psp.tile([P, P], F32, tag="t", bufs=3)
                    nc.tensor.transpose(
                        out=tp[:], in_=s[:, h * D : (h + 1) * D], identity=ident[:]
                    )
                    tps = workp.tile([P, P], F32, tag="tps")
                    nc.vector.tensor_copy(tps[:], tp[:])
                    mm = psp.tile([P, D], F32, tag="m", bufs=3)
                    nc.tensor.matmul(
                        out=mm[:], lhsT=tps[:], rhs=w3t[:], start=True, stop=True
                    )
                    nc.vector.tensor_scalar_mul(
                        e2[:, h * D : (h + 1) * D], mm[:], col(dishhq, t)
                    )
                nc.sync.dma_start(E2h.ap()[rows(t)], e2[:])

            ell_run(E1H, DD, K3, idx3q, None, g3_tail)

            # ---- S11 + S13: realign + scores
            for t in range(nt):
                e2 = gathp.tile([P, DD], F32, tag="gr")
                nc.gpsimd.indirect_dma_start(
                    out=e2[:],
                    out_offset=None,
                    in_=E2h.ap(),
                    in_offset=bass.IndirectOffsetOnAxis(ap=col(idxRq, t), axis=0),
                )
                tv = workp.tile([P, D], F32, tag="sc_tv")
                nc.sync.dma_start(tv[:], TV.ap()[rows(t)])
                pr = workp.tile([P, DD], F32, tag="sc_pr")
                nc.vector.tensor_mul(pr[:, 0:D], tv[:], e2[:, 0:D])
                nc.vector.tensor_mul(pr[:, D:DD], tv[:], e2[:, D:DD])
                rs = workp.tile([P, 2], F32, tag="sc_rs")
                nc.vector.tensor_reduce(
                    out=rs[:],
                    in_=pr[:].rearrange("p (h d) -> p h d", h=2),
                    axis=mybir.AxisListType.X,
                    op=ALU.add,
                )
                sg = outp.tile([P, 2], F32, tag="sc_sg")
                nc.scalar.activation(sg[:], rs[:], AF.Sigmoid)
                nc.sync.dma_start(out.ap()[rows(t), ncls : ncls + 2], sg[:])

            # ---- G4: cls = (dis * sum) @ Wc -> out[:, :ncls]
            def g4_tail(t, s):
                sc_ = workp.tile([P, D], F32, tag="c_s")
                nc.vector.tensor_scalar_mul(sc_[:], s[:], col(disq, t))
                tp = psp.tile([P, P], F32, tag="t", bufs=3)
                nc.tensor.transpose(out=tp[:], in_=sc_[:], identity=ident[:])
                tps = workp.tile([P, P], F32, tag="tps")
                nc.vector.tensor_copy(tps[:], tp[:])
                mm = psp.tile([P, ncls], F32, tag="m", bufs=3)
                nc.tensor.matmul(out=mm[:], lhsT=tps[:], rhs=wct[:], start=True, stop=True)
                o = outp.tile([P, ncls], F32, tag="c_o")
                nc.vector.tensor_copy(o[:], mm[:])
                nc.sync.dma_start(out.ap()[rows(t), 0:ncls], o[:])

            ell_run(E1D, D, K1, idx1q, e1d_s, g4_tail)

    nc.compile()
    return nc


def assemble(results, meta):
    n_cores = meta["n_cores"]
    N = len(meta["core_of"])
    ncls = meta["ncls"]
    out = np.empty((N, ncls + 2), np.float32)
    for c in range(n_cores):
        oc = results[c]["out"]
        m = meta["core_of"] == c
        out[m] = oc[meta["loc_of"][m]]
    return out


# ------------------------------------------------------------------ entry


_CACHE = {}
TRACE = False
LAST_RES = None


def kernel(**inputs):
    """Full-input entry point: shards across 8 NeuronCores internally.

    Expects the nn_MixModel input dict (x, edge_index, edge_index_hop, y,
    perm, W1..Wd); returns the full [N, n_cls+2] float32 output.
    """
    n_cores = 8
    in_maps, meta = prep(inputs, n_cores)
    key = (meta["nloc"], tuple(meta["K1"]), tuple(meta["K3"]))
    nc = _CACHE.get(key)
    if nc is None:
        nc = build(meta)
        _CACHE[key] = nc
    res = bass_utils.run_bass_kernel_spmd(
        nc, in_maps, core_ids=list(range(n_cores)), trace=TRACE
    )
    global LAST_RES
    LAST_RES = res
    return assemble(res.results, meta)


# revision 12
# speedup vs baseline: 1.9360x; 1.0294x over previous
"""8-core Trainium2 Bass kernel for nn_MixModel (GCN mix model) — v3.

Sharding: nodes dealt round-robin by in-degree rank to 8 cores; each core owns
NLOC = ceil((ceil(N/8)+1)/128)*128 local rows (>=1 zero pad row reused as the
ELL gather-pad target).

Algebra used:
 - GCN messages factorize: msg = (h*dis)[src], output scaled by dis[dst]; the
   self-loop term is a local-tile add (pi-order stages) or an extra ELL slot
   (hop stage). Aggregation = unweighted padded-ELL gather+sum of pre-scaled
   table rows.
 - segsum and the layer matmul commute: sum((z@W*dis)[src]) =
   sum((z*dis)[src]) @ W — so cores AllGather the *scaled activations* and the
   per-layer matmul runs on the 98 aggregated dst tiles.
 - good/bad paths share edge sets -> gather concatenated 256-wide tables.
 - the permuted-input path's first-layer table is a cheap local permutation
   gather of the xW1' table (12.5k rows), not a per-edge pass.

Gather engine: gpsimd.dma_gather (InstDMAGatherAnt, mlp ucode library) with
int16 indices. The 100352-row shared tables exceed int16 range, so each ELL
tile is split into 4 chunk rectangles (chunk = 25088 consecutive table rows =
one core pair); chunk-local indices fit int16. Rectangles of consecutive tiles
are batched into one dma_gather call per chunk (<=32 j-columns per call,
<=96 per batch) amortizing the ~1us SWDGE fixed overhead over thousands of
row descriptors. Pad slots point at the chunk's zero pad row (local nloc-1).

Stages (per core):
  S0   xW1' shard = (x_sh @ W1) * dis_sh
  AG0  AllGather -> XW [NG,128]
  S2   T1 shard = [xW1'_loc | gather(XW, gperm)*ratio] ; AG1 -> T1 [NG,256]
  G1   ELL gather T1 -> zd = relu(dis^2 * sum)  (= z1*dis)      -> AG2 ZD
  G2   ELL gather ZD -> S ; e1{,b} = relu(dis * (S_h @ W2)) ;
       ship [e1*dish|e1b*dish] -> AG3a E1H ; [e1*dis] -> AG3b E1D ; e1 local
  G3   ELL gather E1H (hop order) -> embed2{,b} = dish * (S_h @ W3) -> E2h
  S12  MLP: embed3 = relu(e1@M1)@M2 ; tvec = embed3@Wd0
  S11  realign E2h to pi order ; scores = sigmoid(rowsum(tvec * e2{,b}))
  G4   ELL gather E1D -> cls = (dis*sum)@Wc -> OUT[:, :10]
"""

import numpy as np

import concourse.bacc as bacc
import concourse.bass as bass
import concourse.mybir as mybir
import concourse.tile as tile
from concourse import bass_utils
from concourse.masks import make_identity

P = 128
F32 = mybir.dt.float32
I32 = mybir.dt.int32
I16 = mybir.dt.int16
AF = mybir.ActivationFunctionType
ALU = mybir.AluOpType
TDT = mybir.dt.bfloat16  # transport/table dtype

# ----------------------------------------------------------------- host prep


def _ell_build(src_g, dst_core, dst_loc, self_g, n_cores, nloc, padrow):
    """Shared-K ELL: returns (K per tile, per-core int32 [P, sum(K)] arrays,
    p-major-global: element [p, koff[t]+k] = slot k of local row t*128+p)."""
    nt = nloc // P
    counts = np.zeros((n_cores, nloc), np.int64)
    np.add.at(counts, (dst_core, dst_loc), 1)
    n_self = 0 if self_g is None else 1
    cmax = counts.reshape(n_cores, nt, P).max(axis=(0, 2))
    K = (cmax + n_self).astype(np.int64)
    order = np.lexsort((dst_loc, dst_core))
    sc, sl, sg = dst_core[order], dst_loc[order], src_g[order]
    key = sc.astype(np.int64) * nloc + sl
    is_start = np.r_[True, key[1:] != key[:-1]] if len(key) else np.array([], bool)
    run_starts = np.flatnonzero(is_start)
    run_len = np.diff(np.r_[run_starts, len(key)])
    pos_in_run = np.arange(len(key)) - np.repeat(run_starts, run_len)
    koff = np.r_[0, np.cumsum(K)]
    sk = int(koff[-1])
    idx_arrs = []
    for c in range(n_cores):
        arr = np.full((P, sk), padrow[c], np.int64)
        m = sc == c
        loc, pos, gidx = sl[m], pos_in_run[m], sg[m]
        t = loc // P
        p = loc % P
        arr[p, koff[t] + pos + n_self] = gidx
        if n_self:
            allt = np.arange(nloc) // P
            allp = np.arange(nloc) % P
            arr[allp, koff[allt]] = self_g[c]
        idx_arrs.append(arr.astype(np.int32))
    return K.tolist(), idx_arrs


def _plane(vals_loc, nt):
    """[nloc] local-row vector -> [P, nt] plane (local row t*128+p -> [p, t])."""
    return np.ascontiguousarray(vals_loc.reshape(nt, P).T)


def prep(inputs, n_cores=8):
    x = np.asarray(inputs["x"], np.float32)
    ei = np.asarray(inputs["edge_index"], np.int64)
    eih = np.asarray(inputs["edge_index_hop"], np.int64)
    perm = np.asarray(inputs["perm"], np.int64)
    W1 = np.asarray(inputs["W1"], np.float32)
    W2 = np.asarray(inputs["W2"], np.float32)
    W3 = np.asarray(inputs["W3"], np.float32)
    M1 = np.asarray(inputs["M1"], np.float32)
    M2 = np.asarray(inputs["M2"], np.float32)
    Wc = np.asarray(inputs["Wc"], np.float32)
    Wd0 = np.asarray(inputs["Wd"], np.float32)[0]
    for bname in ("b1", "b2", "b3", "mb1", "mb2", "bc"):
        assert np.abs(np.asarray(inputs[bname])).max() == 0.0, (
            f"nonzero bias {bname} not supported by this kernel build"
        )

    N, n_feat = x.shape
    D = W1.shape[1]
    ncls = Wc.shape[1]
    max_real = -(-N // n_cores)
    nloc = -(-(max_real + 1) // P) * P
    nt = nloc // P
    ng = n_cores * nloc

    deg = np.bincount(ei[1], minlength=N).astype(np.float32) + 1.0
    degh = np.bincount(eih[1], minlength=N).astype(np.float32) + 1.0
    dis = 1.0 / np.sqrt(deg)
    dish = 1.0 / np.sqrt(degh)

    order = np.argsort(-deg, kind="stable")
    core_of = np.empty(N, np.int64)
    loc_of = np.empty(N, np.int64)
    core_of[order] = np.arange(N) % n_cores
    loc_of[order] = np.arange(N) // n_cores

    # piece-major global table layout: [piece][core][piece-local rows], so
    # each split-AllGather piece lands contiguously in the shared tables.
    NSPL = 8
    pr0 = np.array([(nt * i // NSPL) * P for i in range(NSPL + 1)], np.int64)

    def glmap(c, r):
        p = np.searchsorted(pr0, r, side="right") - 1
        return 8 * pr0[p] + c * (pr0[p + 1] - pr0[p]) + (r - pr0[p])

    gl = glmap(core_of, loc_of)
    padrow = [int(glmap(np.int64(c), np.int64(nloc - 1))) for c in range(n_cores)]

    nat = np.full((n_cores, nloc), -1, np.int64)
    nat[core_of, loc_of] = np.arange(N)

    # hop order: per-core resort by hop degree desc (pads last)
    hkey = np.where(nat >= 0, -degh[np.maximum(nat, 0)], 1.0)
    hord = np.argsort(hkey, axis=1, kind="stable")
    hpos = np.argsort(hord, axis=1)

    allg = glmap(
        np.repeat(np.arange(n_cores), nloc), np.tile(np.arange(nloc), n_cores)
    ).reshape(n_cores, nloc)
    selfg_pi = np.where(nat >= 0, allg, np.array(padrow)[:, None])
    K1, idx1 = _ell_build(
        gl[ei[0]], core_of[ei[1]], loc_of[ei[1]], None, n_cores, nloc, padrow
    )
    selfg_h = np.take_along_axis(selfg_pi, hord, axis=1)
    K3, idx3 = _ell_build(
        gl[eih[0]],
        core_of[eih[1]],
        hpos[core_of[eih[1]], loc_of[eih[1]]],
        selfg_h,
        n_cores,
        nloc,
        padrow,
    )

    in_maps = []
    for c in range(n_cores):
        natc = nat[c]
        real = natc >= 0
        xs = np.zeros((nloc, n_feat), np.float32)
        xs[real] = x[natc[real]]
        dis_c = np.ones(nloc, np.float32)
        dis_c[real] = dis[natc[real]]
        dish_pi = np.ones(nloc, np.float32)
        dish_pi[real] = dish[natc[real]]
        dishh = np.ones(nloc, np.float32)
        hnat = natc[hord[c]]
        hreal = hnat >= 0
        dishh[hreal] = dish[hnat[hreal]]
        gperm = np.full(nloc, padrow[c], np.int64)
        ratio = np.ones(nloc, np.float32)
        pv = perm[natc[real]]
        gperm[real] = gl[pv]
        ratio[real] = dis[natc[real]] / dis[pv]
        in_maps.append(
            {
                "xTb": np.ascontiguousarray(
                    xs.reshape(nt, P, 4, P).transpose(0, 3, 2, 1).reshape(nt * P, 4 * P)
                ),
                "dis_p": _plane(dis_c, nt),
                "dis2_p": _plane(dis_c * dis_c, nt),
                "dishp_p": _plane(dish_pi, nt),
                "dishh_p": _plane(dishh, nt),
                "ratio_p": _plane(ratio, nt),
                "gperm_p": _plane(gperm.astype(np.int32), nt),
                "idxR_p": _plane(hpos[c].astype(np.int32), nt),
                "idx1": idx1[c],
                "idx3": idx3[c],
                "W1": W1,
                "W2": W2,
                "W3": W3,
                "M1": M1,
                "M2": M2,
                "Wd0": Wd0,
                "Wc": np.ascontiguousarray(Wc),
            }
        )

    meta = dict(
        n_cores=n_cores,
        nloc=nloc,
        nt=nt,
        ng=ng,
        n_feat=n_feat,
        D=D,
        ncls=ncls,
        K1=K1,
        K3=K3,
        core_of=core_of,
        loc_of=loc_of,
    )
    return in_maps, meta


# ------------------------------------------------------------- device build


def build(meta):
    n_cores = meta["n_cores"]
    nloc, nt, ng = meta["nloc"], meta["nt"], meta["ng"]
    n_feat, D, ncls = meta["n_feat"], meta["D"], meta["ncls"]
    K1, K3 = meta["K1"], meta["K3"]
    DD = 2 * D
    nfc = n_feat // P
    sk1, sk3 = sum(K1), sum(K3)
    groups = [list(range(n_cores))]

    nc = bacc.Bacc("TRN2", debug=False, num_devices=n_cores)
    shared = "Shared" if n_cores > 4 else "Local"

    xTb = nc.dram_tensor("xTb", [nt * P, 4 * P], F32, kind="ExternalInput")
    dis_p = nc.dram_tensor("dis_p", [P, nt], F32, kind="ExternalInput")
    dis2_p = nc.dram_tensor("dis2_p", [P, nt], F32, kind="ExternalInput")
    dishp_p = nc.dram_tensor("dishp_p", [P, nt], F32, kind="ExternalInput")
    dishh_p = nc.dram_tensor("dishh_p", [P, nt], F32, kind="ExternalInput")
    ratio_p = nc.dram_tensor("ratio_p", [P, nt], F32, kind="ExternalInput")
    gperm_p = nc.dram_tensor("gperm_p", [P, nt], I32, kind="ExternalInput")
    idxR_p = nc.dram_tensor("idxR_p", [P, nt], I32, kind="ExternalInput")
    idx1 = nc.dram_tensor("idx1", [P, sk1], I32, kind="ExternalInput")
    idx3 = nc.dram_tensor("idx3", [P, sk3], I32, kind="ExternalInput")
    W1 = nc.dram_tensor("W1", [n_feat, D], F32, kind="ExternalInput")
    W2 = nc.dram_tensor("W2", [D, D], F32, kind="ExternalInput")
    W3 = nc.dram_tensor("W3", [D, D], F32, kind="ExternalInput")
    M1 = nc.dram_tensor("M1", [D, D], F32, kind="ExternalInput")
    M2 = nc.dram_tensor("M2", [D, D], F32, kind="ExternalInput")
    Wd0 = nc.dram_tensor("Wd0", [D, D], F32, kind="ExternalInput")
    Wc = nc.dram_tensor("Wc", [D, ncls], F32, kind="ExternalInput")
    out = nc.dram_tensor("out", [nloc, ncls + 2], F32, kind="ExternalOutput")

    xw_s = nc.dram_tensor("xw_s", [nloc, D], TDT, kind="Internal")
    XW = nc.dram_tensor("XW", [ng, D], TDT, kind="Internal", addr_space=shared)
    t1_s = nc.dram_tensor("t1_s", [nloc, DD], TDT, kind="Internal")
    T1 = nc.dram_tensor("T1", [ng, DD], TDT, kind="Internal", addr_space=shared)
    zd_s = nc.dram_tensor("zd_s", [nloc, DD], TDT, kind="Internal")
    ZD = nc.dram_tensor("ZD", [ng, DD], TDT, kind="Internal", addr_space=shared)
    e1_s = nc.dram_tensor("e1_s", [nloc, D], F32, kind="Internal")
    e1h_s = nc.dram_tensor("e1h_s", [nloc, DD], TDT, kind="Internal")
    e1d_s = nc.dram_tensor("e1d_s", [nloc, D], TDT, kind="Internal")
    E1H = nc.dram_tensor("E1H", [ng, DD], TDT, kind="Internal", addr_space=shared)
    E1D = nc.dram_tensor("E1D", [ng, D], TDT, kind="Internal", addr_space=shared)
    E2h = nc.dram_tensor("E2h", [nloc, DD], F32, kind="Internal")
    TV = nc.dram_tensor("TV", [nloc, D], F32, kind="Internal")

    with tile.TileContext(nc) as tc:
        with (
            tc.tile_pool(name="const", bufs=1) as constp,
            tc.tile_pool(name="gath", bufs=4) as gathp,
            tc.tile_pool(name="work", bufs=3) as workp,
            tc.tile_pool(name="outp", bufs=3) as outp,
            tc.tile_pool(name="psum", bufs=2, space="PSUM") as psp,
        ):
            ident = constp.tile([P, P], F32)
            make_identity(nc, ident[:])

            # resident planes + indices
            def res(t_dram, w, dt=F32, name=None):
                tl = constp.tile([P, w], dt, name=name)
                nc.sync.dma_start(tl[:], t_dram.ap())
                return tl

            disq = res(dis_p, nt, name="disq")
            dis2q = res(dis2_p, nt, name="dis2q")
            dishpq = res(dishp_p, nt, name="dishpq")
            dishhq = res(dishh_p, nt, name="dishhq")
            ratioq = res(ratio_p, nt, name="ratioq")
            gpermq = res(gperm_p, nt, I32, name="gpermq")
            idxRq = res(idxR_p, nt, I32, name="idxRq")
            idx1q = res(idx1, sk1, I32, name="idx1q")
            idx3q = res(idx3, sk3, I32, name="idx3q")

            w1t = [
                constp.tile([P, D], F32, name=f"w1t_{i}") for i in range(nfc)
            ]
            for i in range(nfc):
                nc.sync.dma_start(w1t[i][:], W1.ap()[i * P : (i + 1) * P])
            w2t = res(W2, D, name="w2t")
            w3t = res(W3, D, name="w3t")
            m1t = res(M1, D, name="m1t")
            m2t = res(M2, D, name="m2t")
            wdt = res(Wd0, D, name="wdt")
            wct = res(Wc, ncls, name="wct")

            def rows(t):
                return slice(t * P, (t + 1) * P)

            def col(plane, t):
                return plane[:, t : t + 1]

            NSPL = 8
            bound = [nt * (i + 1) // NSPL - 1 for i in range(NSPL)]

            def ag_piece(src, dst, piece):
                r0 = (nt * piece // NSPL) * P
                r1 = (nt * (piece + 1) // NSPL) * P
                nc.gpsimd.collective_compute(
                    "AllGather",
                    ALU.bypass,
                    replica_groups=groups,
                    ins=[src[r0:r1].opt()],
                    outs=[dst[n_cores * r0 : n_cores * r1].opt()],
                )

            # ---- S0: xW1' shard
            sp = 0
            for t in range(nt):
                ps = psp.tile([P, D], F32, tag="mm")
                xt = workp.tile([P, nfc * P], F32, tag="xt")
                nc.sync.dma_start(xt[:], xTb.ap()[rows(t)])
                for i in range(nfc):
                    nc.tensor.matmul(
                        out=ps[:],
                        lhsT=xt[:, i * P : (i + 1) * P],
                        rhs=w1t[i][:],
                        start=(i == 0),
                        stop=(i == nfc - 1),
                    )
                o = outp.tile([P, D], TDT, tag="s0")
                nc.vector.tensor_scalar_mul(o[:], ps[:], col(disq, t))
                nc.sync.dma_start(xw_s.ap()[rows(t)], o[:])
                nc.sync.dma_start(t1_s.ap()[rows(t), 0:D], o[:])
                if t == bound[sp]:
                    ag_piece(xw_s, XW, sp)
                    sp += 1

            # ---- S2: T1 shard (bad half; good half written by S0)
            sp = 0
            for t in range(nt):
                g = gathp.tile([P, D], TDT, tag="g2")
                nc.gpsimd.indirect_dma_start(
                    out=g[:],
                    out_offset=None,
                    in_=XW.ap(),
                    in_offset=bass.IndirectOffsetOnAxis(ap=col(gpermq, t), axis=0),
                )
                o = outp.tile([P, D], TDT, tag="s2")
                nc.vector.tensor_scalar_mul(o[:], g[:], col(ratioq, t))
                nc.sync.dma_start(t1_s.ap()[rows(t), D:DD], o[:])
                if t == bound[sp]:
                    ag_piece(t1_s, T1, sp)
                    sp += 1

            # ---- per-slot indirect ELL gather driver -------------------
            # one indirect DMA per (tile, slot): [P,1] offset column gathers
            # 128 rows; slots accumulate via a vector reduce. The self-loop
            # term is a local-tile add (local_s) instead of an ELL slot.
            def ell_run(table, width, Ks, idxq, local_s, tail):
                koff = 0
                for t in range(nt):
                    K = Ks[t]
                    g = gathp.tile([P, K * width], TDT, tag="ge")
                    for k in range(K):
                        nc.gpsimd.indirect_dma_start(
                            out=g[:, k * width : (k + 1) * width],
                            out_offset=None,
                            in_=table.ap(),
                            in_offset=bass.IndirectOffsetOnAxis(
                                ap=idxq[:, koff + k : koff + k + 1], axis=0
                            ),
                        )
                    koff += K
                    s = workp.tile([P, width], F32, tag="se")
                    if K == 1:
                        nc.vector.tensor_copy(s[:], g[:])
                    else:
                        nc.vector.tensor_reduce(
                            out=s[:],
                            in_=g[:].rearrange("p (k d) -> p d k", k=K),
                            axis=mybir.AxisListType.X,
                            op=ALU.add,
                        )
                    if local_s is not None:
                        li = workp.tile([P, width], TDT, tag="sl")
                        nc.sync.dma_start(li[:], local_s.ap()[rows(t)])
                        nc.vector.tensor_tensor(
                            out=s[:], in0=s[:], in1=li[:], op=ALU.add
                        )
                    tail(t, s)

            # ---- G1: zd = relu(dis2 * sum) -> zd_s
            spl = [0]

            def g1_tail(t, s):
                o = outp.tile([P, DD], TDT, tag="ze")
                nc.vector.tensor_scalar(
                    o[:], s[:], col(dis2q, t), 0.0, ALU.mult, ALU.max
                )
                nc.sync.dma_start(zd_s.ap()[rows(t)], o[:])
                if t == bound[spl[0]]:
                    ag_piece(zd_s, ZD, spl[0])
                    spl[0] += 1

            ell_run(T1, DD, K1, idx1q, t1_s, g1_tail)

            # ---- G2: S @ W2, three shipped variants
            def g2_tail(t, s):
                e1h = outp.tile([P, DD], TDT, tag="e1h")
                e1d = outp.tile([P, D], TDT, tag="e1d")
                e1p = outp.tile([P, D], F32, tag="e1p")
                for h in range(2):
                    tp = psp.tile([P, P], F32, tag="t", bufs=3)
                    nc.tensor.transpose(
                        out=tp[:], in_=s[:, h * D : (h + 1) * D], identity=ident[:]
                    )
                    tps = workp.tile([P, P], F32, tag="tps")
                    nc.vector.tensor_copy(tps[:], tp[:])
                    mm = psp.tile([P, D], F32, tag="m", bufs=3)
                    nc.tensor.matmul(
                        out=mm[:], lhsT=tps[:], rhs=w2t[:], start=True, stop=True
                    )
                    # e1 = relu(dis * mm)
                    eh = workp.tile([P, D], F32, tag="eh")
                    nc.vector.tensor_scalar(
                        eh[:], mm[:], col(disq, t), 0.0, ALU.mult, ALU.max
                    )
                    nc.vector.tensor_scalar_mul(
                        e1h[:, h * D : (h + 1) * D], eh[:], col(dishpq, t)
                    )
                    if h == 0:
                        nc.vector.tensor_copy(e1p[:], eh[:])
                        nc.vector.tensor_scalar_mul(e1d[:], eh[:], col(disq, t))
                nc.sync.dma_start(e1_s.ap()[rows(t)], e1p[:])
                nc.sync.dma_start(e1h_s.ap()[rows(t)], e1h[:])
                nc.sync.dma_start(e1d_s.ap()[rows(t)], e1d[:])
                if t == bound[spl[0]]:
                    ag_piece(e1h_s, E1H, spl[0])
                    ag_piece(e1d_s, E1D, spl[0])
                    spl[0] += 1

            spl[0] = 0
            ell_run(ZD, DD, K1, idx1q, zd_s, g2_tail)

            # ---- S12: MLP + tvec (local, overlaps with AG3/G3)
            for t in range(nt):
                et = workp.tile([P, D], F32, tag="ml_in")
                nc.sync.dma_start(et[:], e1_s.ap()[rows(t)])
                tp = psp.tile([P, P], F32, tag="t", bufs=3)
                nc.tensor.transpose(out=tp[:], in_=et[:], identity=ident[:])
                tps = workp.tile([P, P], F32, tag="tps")
                nc.vector.tensor_copy(tps[:], tp[:])
                mm = psp.tile([P, D], F32, tag="m", bufs=3)
                nc.tensor.matmul(out=mm[:], lhsT=tps[:], rhs=m1t[:], start=True, stop=True)
                u = workp.tile([P, D], F32, tag="ml_u")
                nc.scalar.activation(u[:], mm[:], AF.Relu)
                tp2 = psp.tile([P, P], F32, tag="t", bufs=3)
                nc.tensor.transpose(out=tp2[:], in_=u[:], identity=ident[:])
                tps2 = workp.tile([P, P], F32, tag="tps")
                nc.vector.tensor_copy(tps2[:], tp2[:])
                mm2 = psp.tile([P, D], F32, tag="m", bufs=3)
                nc.tensor.matmul(
                    out=mm2[:], lhsT=tps2[:], rhs=m2t[:], start=True, stop=True
                )
                e3 = workp.tile([P, D], F32, tag="ml_e3")
                nc.vector.tensor_copy(e3[:], mm2[:])
                tp3 = psp.tile([P, P], F32, tag="t", bufs=3)
                nc.tensor.transpose(out=tp3[:], in_=e3[:], identity=ident[:])
                tps3 = workp.tile([P, P], F32, tag="tps")
                nc.vector.tensor_copy(tps3[:], tp3[:])
                mm3 = psp.tile([P, D], F32, tag="m", bufs=3)
                nc.tensor.matmul(
                    out=mm3[:], lhsT=tps3[:], rhs=wdt[:], start=True, stop=True
                )
                tv = outp.tile([P, D], F32, tag="ml_tv")
                nc.vector.tensor_copy(tv[:], mm3[:])
                nc.sync.dma_start(TV.ap()[rows(t)], tv[:])

            # ---- G3: embed2{,b} = dishh * (S_h @ W3) -> E2h (hop order)
            def g3_tail(t, s):
                e2 = outp.tile([P, DD], F32, tag="e2")
                for h in range(2):
                    tp = psp.tile([P, P], F32, tag="t", bufs=3)
                    nc.tensor.transpose(
                        out=tp[:], in_=s[:, h * D : (h + 1) * D], identity=ident[:]
                    )
                    tps = workp.tile([P, P], F32, tag="tps")
                    nc.vector.tensor_copy(tps[:], tp[:])
                    mm = psp.tile([P, D], F32, tag="m", bufs=3)
                    nc.tensor.matmul(
                        out=mm[:], lhsT=tps[:], rhs=w3t[:], start=True, stop=True
                    )
                    nc.vector.tensor_scalar_mul(
                        e2[:, h * D : (h + 1) * D], mm[:], col(dishhq, t)
                    )
                nc.sync.dma_start(E2h.ap()[rows(t)], e2[:])

            ell_run(E1H, DD, K3, idx3q, None, g3_tail)

            # ---- S11 + S13: realign + scores
            for t in range(nt):
                e2 = gathp.tile([P, DD], F32, tag="gr")
                nc.gpsimd.indirect_dma_start(
                    out=e2[:],
                    out_offset=None,
                    in_=E2h.ap(),
                    in_offset=bass.IndirectOffsetOnAxis(ap=col(idxRq, t), axis=0),
                )
                tv = workp.tile([P, D], F32, tag="sc_tv")
                nc.sync.dma_start(tv[:], TV.ap()[rows(t)])
                pr = workp.tile([P, DD], F32, tag="sc_pr")
                nc.vector.tensor_mul(pr[:, 0:D], tv[:], e2[:, 0:D])
                nc.vector.tensor_mul(pr[:, D:DD], tv[:], e2[:, D:DD])
                rs = workp.tile([P, 2], F32, tag="sc_rs")
                nc.vector.tensor_reduce(
                    out=rs[:],
                    in_=pr[:].rearrange("p (h d) -> p h d", h=2),
                    axis=mybir.AxisListType.X,
                    op=ALU.add,
                )
                sg = outp.tile([P, 2], F32, tag="sc_sg")
                nc.scalar.activation(sg[:], rs[:], AF.Sigmoid)
                nc.sync.dma_start(out.ap()[rows(t), ncls : ncls + 2], sg[:])

            # ---- G4: cls = (dis * sum) @ Wc -> out[:, :ncls]
            def g4_tail(t, s):
                sc_ = workp.tile([P, D], F32, tag="c_s")
                nc.vector.tensor_scalar_mul(sc_[:], s[:], col(disq, t))
                tp = psp.tile([P, P], F32, tag="t", bufs=3)
                nc.tensor.transpose(out=tp[:], in_=sc_[:], identity=ident[:])
                tps = workp.tile([P, P], F32, tag="tps")
                nc.vector.tensor_copy(tps[:], tp[:])
                mm = psp.tile([P, ncls], F32, tag="m", bufs=3)
                nc.tensor.matmul(out=mm[:], lhsT=tps[:], rhs=wct[:], start=True, stop=True)
                o = outp.tile([P, ncls], F32, tag="c_o")
                nc.vector.tensor_copy(o[:], mm[:])
                nc.sync.dma_start(out.ap()[rows(t), 0:ncls], o[:])

            ell_run(E1D, D, K1, idx1q, e1d_s, g4_tail)

    nc.compile()
    return nc


def assemble(results, meta):
    n_cores = meta["n_cores"]
    N = len(meta["core_of"])
    ncls = meta["ncls"]
    out = np.empty((N, ncls + 2), np.float32)
    for c in range(n_cores):
        oc = results[c]["out"]
        m = meta["core_of"] == c
        out[m] = oc[meta["loc_of"][m]]
    return out


# ------------------------------------------------------------------ entry


_CACHE = {}
TRACE = False
LAST_RES = None


def kernel(**inputs):
    """Full-input entry point: shards across 8 NeuronCores internally.

    Expects the nn_MixModel input dict (x, edge_index, edge_index_hop, y,
    perm, W1..Wd); returns the full [N, n_cls+2] float32 output.
    """
    n_cores = 8
    in_maps, meta = prep(inputs, n_cores)
    key = (meta["nloc"], tuple(meta["K1"]), tuple(meta["K3"]))
    nc = _CACHE.get(key)
    if nc is None:
        nc = build(meta)
        _CACHE[key] = nc
    res = bass_utils.run_bass_kernel_spmd(
        nc, in_maps, core_ids=list(range(n_cores)), trace=TRACE
    )
    global LAST_RES
    LAST_RES = res
    return assemble(res.results, meta)


# revision 13
# speedup vs baseline: 1.9393x; 1.0017x over previous
"""8-core Trainium2 Bass kernel for nn_MixModel (GCN mix model) — v3.

Sharding: nodes dealt round-robin by in-degree rank to 8 cores; each core owns
NLOC = ceil((ceil(N/8)+1)/128)*128 local rows (>=1 zero pad row reused as the
ELL gather-pad target).

Algebra used:
 - GCN messages factorize: msg = (h*dis)[src], output scaled by dis[dst]; the
   self-loop term is a local-tile add (pi-order stages) or an extra ELL slot
   (hop stage). Aggregation = unweighted padded-ELL gather+sum of pre-scaled
   table rows.
 - segsum and the layer matmul commute: sum((z@W*dis)[src]) =
   sum((z*dis)[src]) @ W — so cores AllGather the *scaled activations* and the
   per-layer matmul runs on the 98 aggregated dst tiles.
 - good/bad paths share edge sets -> gather concatenated 256-wide tables.
 - the permuted-input path's first-layer table is a cheap local permutation
   gather of the xW1' table (12.5k rows), not a per-edge pass.

Gather engine: per-slot indirect_dma_start (SWDGE, ~1.1us/call for 128 rows;
measured DSP descriptor-gen is ~7-11ns/row for every SWDGE mechanism, so the
per-slot ELL at 1.07x padding beats int16-chunked dma_gather at 2.5x padding).
The self-loop term of the pi-order stages is a local-tile add instead of an
ELL slot (saves ~300 calls). Shared tables use a piece-major layout
([piece][core][rows], NSPL=8) so each AllGather is split into 8 contiguous
pieces issued as their source tiles complete, hiding collective latency
under the gather stream. x is staged tile-major ([nt*128, 512] blocks) so S0
loads are single 256KB DMAs with 2KB rows spread across all DMA engines.

Stages (per core):
  S0   xW1' shard = (x_sh @ W1) * dis_sh
  AG0  AllGather -> XW [NG,128]
  S2   T1 shard = [xW1'_loc | gather(XW, gperm)*ratio] ; AG1 -> T1 [NG,256]
  G1   ELL gather T1 -> zd = relu(dis^2 * sum)  (= z1*dis)      -> AG2 ZD
  G2   ELL gather ZD -> S ; e1{,b} = relu(dis * (S_h @ W2)) ;
       ship [e1*dish|e1b*dish] -> AG3a E1H ; [e1*dis] -> AG3b E1D ; e1 local
  G3   ELL gather E1H (hop order) -> embed2{,b} = dish * (S_h @ W3) -> E2h
  S12  MLP: embed3 = relu(e1@M1)@M2 ; tvec = embed3@Wd0
  S11  realign E2h to pi order ; scores = sigmoid(rowsum(tvec * e2{,b}))
  G4   ELL gather E1D -> cls = (dis*sum)@Wc -> OUT[:, :10]
"""

import numpy as np

import concourse.bacc as bacc
import concourse.bass as bass
import concourse.mybir as mybir
import concourse.tile as tile
from concourse import bass_utils
from concourse.masks import make_identity

P = 128
F32 = mybir.dt.float32
I32 = mybir.dt.int32
I16 = mybir.dt.int16
AF = mybir.ActivationFunctionType
ALU = mybir.AluOpType
TDT = mybir.dt.bfloat16  # transport/table dtype

# ----------------------------------------------------------------- host prep


def _ell_build(src_g, dst_core, dst_loc, self_g, n_cores, nloc, padrow):
    """Shared-K ELL: returns (K per tile, per-core int32 [P, sum(K)] arrays,
    p-major-global: element [p, koff[t]+k] = slot k of local row t*128+p)."""
    nt = nloc // P
    counts = np.zeros((n_cores, nloc), np.int64)
    np.add.at(counts, (dst_core, dst_loc), 1)
    n_self = 0 if self_g is None else 1
    cmax = counts.reshape(n_cores, nt, P).max(axis=(0, 2))
    K = (cmax + n_self).astype(np.int64)
    order = np.lexsort((dst_loc, dst_core))
    sc, sl, sg = dst_core[order], dst_loc[order], src_g[order]
    key = sc.astype(np.int64) * nloc + sl
    is_start = np.r_[True, key[1:] != key[:-1]] if len(key) else np.array([], bool)
    run_starts = np.flatnonzero(is_start)
    run_len = np.diff(np.r_[run_starts, len(key)])
    pos_in_run = np.arange(len(key)) - np.repeat(run_starts, run_len)
    koff = np.r_[0, np.cumsum(K)]
    sk = int(koff[-1])
    idx_arrs = []
    for c in range(n_cores):
        arr = np.full((P, sk), padrow[c], np.int64)
        m = sc == c
        loc, pos, gidx = sl[m], pos_in_run[m], sg[m]
        t = loc // P
        p = loc % P
        arr[p, koff[t] + pos + n_self] = gidx
        if n_self:
            allt = np.arange(nloc) // P
            allp = np.arange(nloc) % P
            arr[allp, koff[allt]] = self_g[c]
        idx_arrs.append(arr.astype(np.int32))
    return K.tolist(), idx_arrs


def _plane(vals_loc, nt):
    """[nloc] local-row vector -> [P, nt] plane (local row t*128+p -> [p, t])."""
    return np.ascontiguousarray(vals_loc.reshape(nt, P).T)


def prep(inputs, n_cores=8):
    x = np.asarray(inputs["x"], np.float32)
    ei = np.asarray(inputs["edge_index"], np.int64)
    eih = np.asarray(inputs["edge_index_hop"], np.int64)
    perm = np.asarray(inputs["perm"], np.int64)
    W1 = np.asarray(inputs["W1"], np.float32)
    W2 = np.asarray(inputs["W2"], np.float32)
    W3 = np.asarray(inputs["W3"], np.float32)
    M1 = np.asarray(inputs["M1"], np.float32)
    M2 = np.asarray(inputs["M2"], np.float32)
    Wc = np.asarray(inputs["Wc"], np.float32)
    Wd0 = np.asarray(inputs["Wd"], np.float32)[0]
    for bname in ("b1", "b2", "b3", "mb1", "mb2", "bc"):
        assert np.abs(np.asarray(inputs[bname])).max() == 0.0, (
            f"nonzero bias {bname} not supported by this kernel build"
        )

    N, n_feat = x.shape
    D = W1.shape[1]
    ncls = Wc.shape[1]
    max_real = -(-N // n_cores)
    nloc = -(-(max_real + 1) // P) * P
    nt = nloc // P
    ng = n_cores * nloc

    deg = np.bincount(ei[1], minlength=N).astype(np.float32) + 1.0
    degh = np.bincount(eih[1], minlength=N).astype(np.float32) + 1.0
    dis = 1.0 / np.sqrt(deg)
    dish = 1.0 / np.sqrt(degh)

    order = np.argsort(-deg, kind="stable")
    core_of = np.empty(N, np.int64)
    loc_of = np.empty(N, np.int64)
    core_of[order] = np.arange(N) % n_cores
    loc_of[order] = np.arange(N) // n_cores

    # piece-major global table layout: [piece][core][piece-local rows], so
    # each split-AllGather piece lands contiguously in the shared tables.
    NSPL = 8
    pr0 = np.array([(nt * i // NSPL) * P for i in range(NSPL + 1)], np.int64)

    def glmap(c, r):
        p = np.searchsorted(pr0, r, side="right") - 1
        return 8 * pr0[p] + c * (pr0[p + 1] - pr0[p]) + (r - pr0[p])

    gl = glmap(core_of, loc_of)
    padrow = [int(glmap(np.int64(c), np.int64(nloc - 1))) for c in range(n_cores)]

    nat = np.full((n_cores, nloc), -1, np.int64)
    nat[core_of, loc_of] = np.arange(N)

    # hop order: per-core resort by hop degree desc (pads last)
    hkey = np.where(nat >= 0, -degh[np.maximum(nat, 0)], 1.0)
    hord = np.argsort(hkey, axis=1, kind="stable")
    hpos = np.argsort(hord, axis=1)

    allg = glmap(
        np.repeat(np.arange(n_cores), nloc), np.tile(np.arange(nloc), n_cores)
    ).reshape(n_cores, nloc)
    selfg_pi = np.where(nat >= 0, allg, np.array(padrow)[:, None])
    K1, idx1 = _ell_build(
        gl[ei[0]], core_of[ei[1]], loc_of[ei[1]], None, n_cores, nloc, padrow
    )
    selfg_h = np.take_along_axis(selfg_pi, hord, axis=1)
    K3, idx3 = _ell_build(
        gl[eih[0]],
        core_of[eih[1]],
        hpos[core_of[eih[1]], loc_of[eih[1]]],
        selfg_h,
        n_cores,
        nloc,
        padrow,
    )

    in_maps = []
    for c in range(n_cores):
        natc = nat[c]
        real = natc >= 0
        xs = np.zeros((nloc, n_feat), np.float32)
        xs[real] = x[natc[real]]
        dis_c = np.ones(nloc, np.float32)
        dis_c[real] = dis[natc[real]]
        dish_pi = np.ones(nloc, np.float32)
        dish_pi[real] = dish[natc[real]]
        dishh = np.ones(nloc, np.float32)
        hnat = natc[hord[c]]
        hreal = hnat >= 0
        dishh[hreal] = dish[hnat[hreal]]
        gperm = np.full(nloc, padrow[c], np.int64)
        ratio = np.ones(nloc, np.float32)
        pv = perm[natc[real]]
        gperm[real] = gl[pv]
        ratio[real] = dis[natc[real]] / dis[pv]
        in_maps.append(
            {
                "xTb": np.ascontiguousarray(
                    xs.reshape(nt, P, 4, P).transpose(0, 3, 2, 1).reshape(nt * P, 4 * P)
                ),
                "dis_p": _plane(dis_c, nt),
                "dis2_p": _plane(dis_c * dis_c, nt),
                "dishp_p": _plane(dish_pi, nt),
                "dishh_p": _plane(dishh, nt),
                "ratio_p": _plane(ratio, nt),
                "gperm_p": _plane(gperm.astype(np.int32), nt),
                "idxR_p": _plane(hpos[c].astype(np.int32), nt),
                "idx1": idx1[c],
                "idx3": idx3[c],
                "W1": W1,
                "W2": W2,
                "W3": W3,
                "M1": M1,
                "M2": M2,
                "Wd0": Wd0,
                "Wc": np.ascontiguousarray(Wc),
            }
        )

    meta = dict(
        n_cores=n_cores,
        nloc=nloc,
        nt=nt,
        ng=ng,
        n_feat=n_feat,
        D=D,
        ncls=ncls,
        K1=K1,
        K3=K3,
        core_of=core_of,
        loc_of=loc_of,
    )
    return in_maps, meta


# ------------------------------------------------------------- device build


def build(meta):
    n_cores = meta["n_cores"]
    nloc, nt, ng = meta["nloc"], meta["nt"], meta["ng"]
    n_feat, D, ncls = meta["n_feat"], meta["D"], meta["ncls"]
    K1, K3 = meta["K1"], meta["K3"]
    DD = 2 * D
    nfc = n_feat // P
    sk1, sk3 = sum(K1), sum(K3)
    groups = [list(range(n_cores))]

    nc = bacc.Bacc("TRN2", debug=False, num_devices=n_cores)
    shared = "Shared" if n_cores > 4 else "Local"

    xTb = nc.dram_tensor("xTb", [nt * P, 4 * P], F32, kind="ExternalInput")
    dis_p = nc.dram_tensor("dis_p", [P, nt], F32, kind="ExternalInput")
    dis2_p = nc.dram_tensor("dis2_p", [P, nt], F32, kind="ExternalInput")
    dishp_p = nc.dram_tensor("dishp_p", [P, nt], F32, kind="ExternalInput")
    dishh_p = nc.dram_tensor("dishh_p", [P, nt], F32, kind="ExternalInput")
    ratio_p = nc.dram_tensor("ratio_p", [P, nt], F32, kind="ExternalInput")
    gperm_p = nc.dram_tensor("gperm_p", [P, nt], I32, kind="ExternalInput")
    idxR_p = nc.dram_tensor("idxR_p", [P, nt], I32, kind="ExternalInput")
    idx1 = nc.dram_tensor("idx1", [P, sk1], I32, kind="ExternalInput")
    idx3 = nc.dram_tensor("idx3", [P, sk3], I32, kind="ExternalInput")
    W1 = nc.dram_tensor("W1", [n_feat, D], F32, kind="ExternalInput")
    W2 = nc.dram_tensor("W2", [D, D], F32, kind="ExternalInput")
    W3 = nc.dram_tensor("W3", [D, D], F32, kind="ExternalInput")
    M1 = nc.dram_tensor("M1", [D, D], F32, kind="ExternalInput")
    M2 = nc.dram_tensor("M2", [D, D], F32, kind="ExternalInput")
    Wd0 = nc.dram_tensor("Wd0", [D, D], F32, kind="ExternalInput")
    Wc = nc.dram_tensor("Wc", [D, ncls], F32, kind="ExternalInput")
    out = nc.dram_tensor("out", [nloc, ncls + 2], F32, kind="ExternalOutput")

    xw_s = nc.dram_tensor("xw_s", [nloc, D], TDT, kind="Internal")
    XW = nc.dram_tensor("XW", [ng, D], TDT, kind="Internal", addr_space=shared)
    t1_s = nc.dram_tensor("t1_s", [nloc, DD], TDT, kind="Internal")
    T1 = nc.dram_tensor("T1", [ng, DD], TDT, kind="Internal", addr_space=shared)
    zd_s = nc.dram_tensor("zd_s", [nloc, DD], TDT, kind="Internal")
    ZD = nc.dram_tensor("ZD", [ng, DD], TDT, kind="Internal", addr_space=shared)
    e1_s = nc.dram_tensor("e1_s", [nloc, D], F32, kind="Internal")
    e1h_s = nc.dram_tensor("e1h_s", [nloc, DD], TDT, kind="Internal")
    e1d_s = nc.dram_tensor("e1d_s", [nloc, D], TDT, kind="Internal")
    E1H = nc.dram_tensor("E1H", [ng, DD], TDT, kind="Internal", addr_space=shared)
    E1D = nc.dram_tensor("E1D", [ng, D], TDT, kind="Internal", addr_space=shared)
    E2h = nc.dram_tensor("E2h", [nloc, DD], F32, kind="Internal")
    TV = nc.dram_tensor("TV", [nloc, D], F32, kind="Internal")

    with tile.TileContext(nc) as tc:
        with (
            tc.tile_pool(name="const", bufs=1) as constp,
            tc.tile_pool(name="gath", bufs=4) as gathp,
            tc.tile_pool(name="work", bufs=3) as workp,
            tc.tile_pool(name="outp", bufs=3) as outp,
            tc.tile_pool(name="psum", bufs=2, space="PSUM") as psp,
        ):
            ident = constp.tile([P, P], F32)
            make_identity(nc, ident[:])

            # resident planes + indices
            def res(t_dram, w, dt=F32, name=None):
                tl = constp.tile([P, w], dt, name=name)
                nc.sync.dma_start(tl[:], t_dram.ap())
                return tl

            disq = res(dis_p, nt, name="disq")
            dis2q = res(dis2_p, nt, name="dis2q")
            dishpq = res(dishp_p, nt, name="dishpq")
            dishhq = res(dishh_p, nt, name="dishhq")
            ratioq = res(ratio_p, nt, name="ratioq")
            gpermq = res(gperm_p, nt, I32, name="gpermq")
            idxRq = res(idxR_p, nt, I32, name="idxRq")
            idx1q = res(idx1, sk1, I32, name="idx1q")
            idx3q = res(idx3, sk3, I32, name="idx3q")

            w1t = [
                constp.tile([P, D], F32, name=f"w1t_{i}") for i in range(nfc)
            ]
            for i in range(nfc):
                nc.sync.dma_start(w1t[i][:], W1.ap()[i * P : (i + 1) * P])
            w2t = res(W2, D, name="w2t")
            w3t = res(W3, D, name="w3t")
            m1t = res(M1, D, name="m1t")
            m2t = res(M2, D, name="m2t")
            wdt = res(Wd0, D, name="wdt")
            wct = res(Wc, ncls, name="wct")

            def rows(t):
                return slice(t * P, (t + 1) * P)

            def col(plane, t):
                return plane[:, t : t + 1]

            NSPL = 8
            bound = [nt * (i + 1) // NSPL - 1 for i in range(NSPL)]

            def ag_piece(src, dst, piece):
                r0 = (nt * piece // NSPL) * P
                r1 = (nt * (piece + 1) // NSPL) * P
                nc.gpsimd.collective_compute(
                    "AllGather",
                    ALU.bypass,
                    replica_groups=groups,
                    ins=[src[r0:r1].opt()],
                    outs=[dst[n_cores * r0 : n_cores * r1].opt()],
                )

            # ---- S0: xW1' shard
            sp = 0
            for t in range(nt):
                ps = psp.tile([P, D], F32, tag="mm")
                xt = workp.tile([P, nfc * P], F32, tag="xt")
                nc.sync.dma_start(xt[:], xTb.ap()[rows(t)])
                for i in range(nfc):
                    nc.tensor.matmul(
                        out=ps[:],
                        lhsT=xt[:, i * P : (i + 1) * P],
                        rhs=w1t[i][:],
                        start=(i == 0),
                        stop=(i == nfc - 1),
                    )
                o = outp.tile([P, D], TDT, tag="s0")
                nc.vector.tensor_scalar_mul(o[:], ps[:], col(disq, t))
                nc.sync.dma_start(xw_s.ap()[rows(t)], o[:])
                nc.sync.dma_start(t1_s.ap()[rows(t), 0:D], o[:])
                if t == bound[sp]:
                    ag_piece(xw_s, XW, sp)
                    sp += 1

            # ---- S2: T1 shard (bad half; good half written by S0)
            sp = 0
            for t in range(nt):
                g = gathp.tile([P, D], TDT, tag="g2")
                nc.gpsimd.indirect_dma_start(
                    out=g[:],
                    out_offset=None,
                    in_=XW.ap(),
                    in_offset=bass.IndirectOffsetOnAxis(ap=col(gpermq, t), axis=0),
                )
                o = outp.tile([P, D], TDT, tag="s2")
                nc.vector.tensor_scalar_mul(o[:], g[:], col(ratioq, t))
                nc.sync.dma_start(t1_s.ap()[rows(t), D:DD], o[:])
                if t == bound[sp]:
                    ag_piece(t1_s, T1, sp)
                    sp += 1

            # ---- per-slot indirect ELL gather driver -------------------
            # one indirect DMA per (tile, slot): [P,1] offset column gathers
            # 128 rows; slots accumulate via a vector reduce. The self-loop
            # term is a local-tile add (local_s) instead of an ELL slot.
            def ell_run(table, width, Ks, idxq, local_s, tail):
                koff = 0
                for t in range(nt):
                    K = Ks[t]
                    g = gathp.tile([P, K * width], TDT, tag="ge")
                    for k in range(K):
                        nc.gpsimd.indirect_dma_start(
                            out=g[:, k * width : (k + 1) * width],
                            out_offset=None,
                            in_=table.ap(),
                            in_offset=bass.IndirectOffsetOnAxis(
                                ap=idxq[:, koff + k : koff + k + 1], axis=0
                            ),
                        )
                    koff += K
                    s = workp.tile([P, width], F32, tag="se")
                    if K == 1:
                        nc.vector.tensor_copy(s[:], g[:])
                    else:
                        nc.vector.tensor_reduce(
                            out=s[:],
                            in_=g[:].rearrange("p (k d) -> p d k", k=K),
                            axis=mybir.AxisListType.X,
                            op=ALU.add,
                        )
                    if local_s is not None:
                        li = workp.tile([P, width], TDT, tag="sl")
                        nc.sync.dma_start(li[:], local_s.ap()[rows(t)])
                        nc.vector.tensor_tensor(
                            out=s[:], in0=s[:], in1=li[:], op=ALU.add
                        )
                    tail(t, s)

            # ---- G1: zd = relu(dis2 * sum) -> zd_s
            spl = [0]

            def g1_tail(t, s):
                o = outp.tile([P, DD], TDT, tag="ze")
                nc.vector.tensor_scalar(
                    o[:], s[:], col(dis2q, t), 0.0, ALU.mult, ALU.max
                )
                nc.sync.dma_start(zd_s.ap()[rows(t)], o[:])
                if t == bound[spl[0]]:
                    ag_piece(zd_s, ZD, spl[0])
                    spl[0] += 1

            ell_run(T1, DD, K1, idx1q, t1_s, g1_tail)

            # ---- G2: S @ W2, three shipped variants
            def g2_tail(t, s):
                e1h = outp.tile([P, DD], TDT, tag="e1h")
                e1d = outp.tile([P, D], TDT, tag="e1d")
                e1p = outp.tile([P, D], F32, tag="e1p")
                for h in range(2):
                    tp = psp.tile([P, P], F32, tag="t", bufs=3)
                    nc.tensor.transpose(
                        out=tp[:], in_=s[:, h * D : (h + 1) * D], identity=ident[:]
                    )
                    tps = workp.tile([P, P], F32, tag="tps")
                    nc.vector.tensor_copy(tps[:], tp[:])
                    mm = psp.tile([P, D], F32, tag="m", bufs=3)
                    nc.tensor.matmul(
                        out=mm[:], lhsT=tps[:], rhs=w2t[:], start=True, stop=True
                    )
                    # e1 = relu(dis * mm)
                    eh = workp.tile([P, D], F32, tag="eh")
                    nc.vector.tensor_scalar(
                        eh[:], mm[:], col(disq, t), 0.0, ALU.mult, ALU.max
                    )
                    nc.vector.tensor_scalar_mul(
                        e1h[:, h * D : (h + 1) * D], eh[:], col(dishpq, t)
                    )
                    if h == 0:
                        nc.vector.tensor_copy(e1p[:], eh[:])
                        nc.vector.tensor_scalar_mul(e1d[:], eh[:], col(disq, t))
                nc.sync.dma_start(e1_s.ap()[rows(t)], e1p[:])
                nc.sync.dma_start(e1h_s.ap()[rows(t)], e1h[:])
                nc.sync.dma_start(e1d_s.ap()[rows(t)], e1d[:])
                if t == bound[spl[0]]:
                    ag_piece(e1h_s, E1H, spl[0])
                    ag_piece(e1d_s, E1D, spl[0])
                    spl[0] += 1

            spl[0] = 0
            ell_run(ZD, DD, K1, idx1q, zd_s, g2_tail)

            # ---- S12: MLP + tvec (local, overlaps with AG3/G3)
            for t in range(nt):
                et = workp.tile([P, D], F32, tag="ml_in")
                nc.sync.dma_start(et[:], e1_s.ap()[rows(t)])
                tp = psp.tile([P, P], F32, tag="t", bufs=3)
                nc.tensor.transpose(out=tp[:], in_=et[:], identity=ident[:])
                tps = workp.tile([P, P], F32, tag="tps")
                nc.vector.tensor_copy(tps[:], tp[:])
                mm = psp.tile([P, D], F32, tag="m", bufs=3)
                nc.tensor.matmul(out=mm[:], lhsT=tps[:], rhs=m1t[:], start=True, stop=True)
                u = workp.tile([P, D], F32, tag="ml_u")
                nc.scalar.activation(u[:], mm[:], AF.Relu)
                tp2 = psp.tile([P, P], F32, tag="t", bufs=3)
                nc.tensor.transpose(out=tp2[:], in_=u[:], identity=ident[:])
                tps2 = workp.tile([P, P], F32, tag="tps")
                nc.vector.tensor_copy(tps2[:], tp2[:])
                mm2 = psp.tile([P, D], F32, tag="m", bufs=3)
                nc.tensor.matmul(
                    out=mm2[:], lhsT=tps2[:], rhs=m2t[:], start=True, stop=True
                )
                e3 = workp.tile([P, D], F32, tag="ml_e3")
                nc.vector.tensor_copy(e3[:], mm2[:])
                tp3 = psp.tile([P, P], F32, tag="t", bufs=3)
                nc.tensor.transpose(out=tp3[:], in_=e3[:], identity=ident[:])
                tps3 = workp.tile([P, P], F32, tag="tps")
                nc.vector.tensor_copy(tps3[:], tp3[:])
                mm3 = psp.tile([P, D], F32, tag="m", bufs=3)
                nc.tensor.matmul(
                    out=mm3[:], lhsT=tps3[:], rhs=wdt[:], start=True, stop=True
                )
                tv = outp.tile([P, D], F32, tag="ml_tv")
                nc.vector.tensor_copy(tv[:], mm3[:])
                nc.sync.dma_start(TV.ap()[rows(t)], tv[:])

            # ---- G3: embed2{,b} = dishh * (S_h @ W3) -> E2h (hop order)
            def g3_tail(t, s):
                e2 = outp.tile([P, DD], F32, tag="e2")
                for h in range(2):
                    tp = psp.tile([P, P], F32, tag="t", bufs=3)
                    nc.tensor.transpose(
                        out=tp[:], in_=s[:, h * D : (h + 1) * D], identity=ident[:]
                    )
                    tps = workp.tile([P, P], F32, tag="tps")
                    nc.vector.tensor_copy(tps[:], tp[:])
                    mm = psp.tile([P, D], F32, tag="m", bufs=3)
                    nc.tensor.matmul(
                        out=mm[:], lhsT=tps[:], rhs=w3t[:], start=True, stop=True
                    )
                    nc.vector.tensor_scalar_mul(
                        e2[:, h * D : (h + 1) * D], mm[:], col(dishhq, t)
                    )
                nc.sync.dma_start(E2h.ap()[rows(t)], e2[:])

            ell_run(E1H, DD, K3, idx3q, None, g3_tail)

            # ---- S11 + S13: realign + scores
            for t in range(nt):
                e2 = gathp.tile([P, DD], F32, tag="gr")
                nc.gpsimd.indirect_dma_start(
                    out=e2[:],
                    out_offset=None,
                    in_=E2h.ap(),
                    in_offset=bass.IndirectOffsetOnAxis(ap=col(idxRq, t), axis=0),
                )
                tv = workp.tile([P, D], F32, tag="sc_tv")
                nc.sync.dma_start(tv[:], TV.ap()[rows(t)])
                pr = workp.tile([P, DD], F32, tag="sc_pr")
                nc.vector.tensor_mul(pr[:, 0:D], tv[:], e2[:, 0:D])
                nc.vector.tensor_mul(pr[:, D:DD], tv[:], e2[:, D:DD])
                rs = workp.tile([P, 2], F32, tag="sc_rs")
                nc.vector.tensor_reduce(
                    out=rs[:],
                    in_=pr[:].rearrange("p (h d) -> p h d", h=2),
                    axis=mybir.AxisListType.X,
                    op=ALU.add,
                )
                sg = outp.tile([P, 2], F32, tag="sc_sg")
                nc.scalar.activation(sg[:], rs[:], AF.Sigmoid)
                nc.sync.dma_start(out.ap()[rows(t), ncls : ncls + 2], sg[:])

            # ---- G4: cls = (dis * sum) @ Wc -> out[:, :ncls]
            def g4_tail(t, s):
                sc_ = workp.tile([P, D], F32, tag="c_s")
                nc.vector.tensor_scalar_mul(sc_[:], s[:], col(disq, t))
                tp = psp.tile([P, P], F32, tag="t", bufs=3)
                nc.tensor.transpose(out=tp[:], in_=sc_[:], identity=ident[:])
                tps = workp.tile([P, P], F32, tag="tps")
                nc.vector.tensor_copy(tps[:], tp[:])
                mm = psp.tile([P, ncls], F32, tag="m", bufs=3)
                nc.tensor.matmul(out=mm[:], lhsT=tps[:], rhs=wct[:], start=True, stop=True)
                o = outp.tile([P, ncls], F32, tag="c_o")
                nc.vector.tensor_copy(o[:], mm[:])
                nc.sync.dma_start(out.ap()[rows(t), 0:ncls], o[:])

            ell_run(E1D, D, K1, idx1q, e1d_s, g4_tail)

    nc.compile()
    return nc


def assemble(results, meta):
    n_cores = meta["n_cores"]
    N = len(meta["core_of"])
    ncls = meta["ncls"]
    out = np.empty((N, ncls + 2), np.float32)
    for c in range(n_cores):
        oc = results[c]["out"]
        m = meta["core_of"] == c
        out[m] = oc[meta["loc_of"][m]]
    return out


# ------------------------------------------------------------------ entry


_CACHE = {}
TRACE = False
LAST_RES = None


def kernel(**inputs):
    """Full-input entry point: shards across 8 NeuronCores internally.

    Expects the nn_MixModel input dict (x, edge_index, edge_index_hop, y,
    perm, W1..Wd); returns the full [N, n_cls+2] float32 output.
    """
    n_cores = 8
    in_maps, meta = prep(inputs, n_cores)
    key = (meta["nloc"], tuple(meta["K1"]), tuple(meta["K3"]))
    nc = _CACHE.get(key)
    if nc is None:
        nc = build(meta)
        _CACHE[key] = nc
    res = bass_utils.run_bass_kernel_spmd(
        nc, in_maps, core_ids=list(range(n_cores)), trace=TRACE
    )
    global LAST_RES
    LAST_RES = res
    return assemble(res.results, meta)
